# revision 10
# baseline (speedup 1.0000x reference)
import sys

for p in ('/opt/trn_rl_repo', '/root/problem'):
    if p not in sys.path:
        sys.path.insert(0, p)

import numpy as np

# ---- problem constants (hardcoded per contract) ----
B, N, F_, C = 4, 30000, 60000, 128
NNZ = 720000
EPS = 1e-5
NC = 8
NPC = 3840                  # nodes per core (8*3840 = 30720 >= 30000), /128
FPC = 7552                  # faces per core (8*7552 = 60416 >= 60000), /128
NR1 = FPC * 4               # dest rows per core, phase 1
NR2 = NPC * 4               # dest rows per core, phase 3
NB1 = NR1 // 128            # 236
NB2 = NR2 // 128            # 120
T1_ROWS = NC * NPC * 4      # 122880
T2_ROWS = NC * FPC * 4      # 241664
W1 = 4
W2 = 8
WIN = 32768
CH1 = 16                    # slots per gather chunk (2048 entries), phase 1
CH2 = 8                     # slots per gather chunk (1024 entries), phase 3


def _schedule(dst_core, dst_rel, src_row, vals, nblocks, nwin):
    blk = dst_rel // 128
    rel = (dst_rel % 128).astype(np.float32)
    win = src_row // WIN
    idx16 = (src_row - win * WIN).astype(np.int16)

    counts = np.zeros((NC, nblocks, nwin), dtype=np.int64)
    np.add.at(counts, (dst_core, blk, win), 1)
    need = (counts.max(axis=0) + 127) // 128
    need = np.maximum(need, 1)
    slots = [[w for w in range(nwin) for _ in range(int(need[b, w]))]
             for b in range(nblocks)]
    n_slots_w = [int(need[:, w].sum()) for w in range(nwin)]
    base_w = []
    for w in range(nwin):
        base = np.zeros(nblocks + 1, dtype=np.int64)
        pos = 0
        for b in range(nblocks):
            base[b] = pos
            pos += int(need[b, w]) * 128
        base[nblocks] = pos
        base_w.append(base)

    per_core = []
    for c in range(NC):
        m = dst_core == c
        cb, cw = blk[m], win[m]
        order = np.lexsort((cw, cb))
        cb, cw = cb[order], cw[order]
        cr, cv, ci = rel[m][order], vals[m][order], idx16[m][order]
        streams = []
        for w in range(nwin):
            ns = n_slots_w[w]
            si = np.zeros(ns * 128, dtype=np.int16)
            sr = np.zeros(ns * 128, dtype=np.float32)
            sv = np.zeros(ns * 128, dtype=np.float32)
            sel = cw == w
            eb = cb[sel]
            off = np.zeros(len(eb), dtype=np.int64)
            if len(eb):
                change = np.flatnonzero(np.diff(eb) != 0) + 1
                starts = np.concatenate(([0], change))
                lens = np.diff(np.concatenate((starts, [len(eb)])))
                off = np.arange(len(eb)) - np.repeat(starts, lens)
            dst = base_w[w][eb] + off
            si[dst] = ci[sel]
            sr[dst] = cr[sel]
            sv[dst] = cv[sel]
            streams.append((si, sr, sv))
        per_core.append(streams)
    return slots, n_slots_w, per_core


def _wrap16(si, n_slots, chunk_slots):
    ns_pad = max(((n_slots + chunk_slots - 1) // chunk_slots) * chunk_slots, chunk_slots)
    full = np.zeros(ns_pad * 128, dtype=np.int16)
    full[:len(si)] = si
    cols = ns_pad * 8
    w = np.zeros((128, cols), dtype=np.int16)
    idx = np.arange(ns_pad * 128)
    w[idx % 16, idx // 16] = full
    for g in range(1, 8):
        w[g * 16:(g + 1) * 16] = w[:16]
    return w, ns_pad


def _slotpack(arr, n_slots):
    out = np.zeros((128, max(n_slots, 1)), dtype=np.float32)
    if n_slots:
        out[:, :n_slots] = arr.reshape(n_slots, 128).T
    return out


def kernel(Di_rows, Di_cols, Di_vals, DiA_rows, DiA_cols, DiA_vals, v, f,
           bn0_gamma, bn0_beta, fc0_w, fc0_b, bn1_gamma, bn1_beta, fc1_w, fc1_b):
    import concourse.bass as bass
    import concourse.bacc as bacc
    import concourse.tile as tile
    from concourse import mybir
    from concourse.bass_utils import run_bass_kernel_spmd

    f32 = mybir.dt.float32
    AF = mybir.ActivationFunctionType
    OP = mybir.AluOpType

    Di_rows = np.asarray(Di_rows); Di_cols = np.asarray(Di_cols)
    Di_vals = np.asarray(Di_vals, dtype=np.float32)
    DiA_rows = np.asarray(DiA_rows); DiA_cols = np.asarray(DiA_cols)
    DiA_vals = np.asarray(DiA_vals, dtype=np.float32)
    v = np.asarray(v, dtype=np.float32); f = np.asarray(f, dtype=np.float32)
    bn0_gamma = np.asarray(bn0_gamma, dtype=np.float32)
    bn0_beta = np.asarray(bn0_beta, dtype=np.float32)
    fc0_w = np.asarray(fc0_w, dtype=np.float32)
    fc0_b = np.asarray(fc0_b, dtype=np.float32)
    bn1_gamma = np.asarray(bn1_gamma, dtype=np.float32)
    bn1_beta = np.asarray(bn1_beta, dtype=np.float32)
    fc1_w = np.asarray(fc1_w, dtype=np.float32)
    fc1_b = np.asarray(fc1_b, dtype=np.float32)

    # ---------------- host-side index preprocessing ----------------
    r = Di_rows.astype(np.int64)
    fc = r // 4
    d1_core = fc // FPC
    d1_rel = 4 * (fc - d1_core * FPC) + (r % 4)
    c4 = Di_cols.astype(np.int64)
    nd = c4 // 4
    scr = nd // NPC
    t1row = scr * (NPC * 4) + 4 * (nd - scr * NPC) + (c4 % 4)
    slots1, nsl1, pc1 = _schedule(d1_core, d1_rel, t1row, Di_vals, NB1, W1)

    r2 = DiA_rows.astype(np.int64)
    nd2 = r2 // 4
    d2_core = nd2 // NPC
    d2_rel = 4 * (nd2 - d2_core * NPC) + (r2 % 4)
    c42 = DiA_cols.astype(np.int64)
    fc2 = c42 // 4
    sc2 = fc2 // FPC
    t2row = sc2 * (FPC * 4) + 4 * (fc2 - sc2 * FPC) + (c42 % 4)
    slots2, nsl2, pc2 = _schedule(d2_core, d2_rel, t2row, DiA_vals, NB2, W2)

    meta = []
    ncols1 = [None] * W1
    ncols2 = [None] * W2
    for c in range(NC):
        m = {}
        for w in range(W1):
            si, sr, sv = pc1[c][w]
            wi, ns_pad = _wrap16(si, nsl1[w], CH1)
            ncols1[w] = ns_pad
            m[f"p1idx{w}"] = wi
            m[f"p1rel{w}"] = _slotpack(sr, nsl1[w])
            m[f"p1val{w}"] = _slotpack(sv, nsl1[w])
        for w in range(W2):
            si, sr, sv = pc2[c][w]
            wi, ns_pad = _wrap16(si, nsl2[w], CH2)
            ncols2[w] = ns_pad
            m[f"p3idx{w}"] = wi
            m[f"p3rel{w}"] = _slotpack(sr, nsl2[w])
            m[f"p3val{w}"] = _slotpack(sv, nsl2[w])
        meta.append(m)

    vpad = np.zeros((B, NC * NPC, C), dtype=np.float32)
    vpad[:, :N] = v
    fpad = np.zeros((B, NC * FPC, C), dtype=np.float32)
    fpad[:, :F_] = f

    kk = np.arange(32); ss = np.arange(4)
    perm_ks = (128 + 32 * ss[None, :] + kk[:, None]).reshape(-1)
    W0T = fc0_w.T
    W1T = fc1_w.T
    consts = {
        "w0f": W0T[:128].copy(), "w0x": W0T[perm_ks].copy(),
        "w1f": W1T[:128].copy(), "w1x": W1T[perm_ks].copy(),
        "bn0v": np.stack([bn0_gamma[:128], bn0_beta[:128],
                          bn0_gamma[perm_ks], bn0_beta[perm_ks]]),
        "bn1v": np.stack([bn1_gamma[:128], bn1_beta[:128],
                          bn1_gamma[perm_ks], bn1_beta[perm_ks]]),
        "fb0": fc0_b.reshape(1, 128).copy(), "fb1": fc1_b.reshape(1, 128).copy(),
        "iota": np.tile(np.arange(128, dtype=np.float32), (128, 1)),
        "ident": np.eye(128, dtype=np.float32),
        "onesc": np.ones((128, 1), dtype=np.float32),
        "onesr": np.ones((1, 128), dtype=np.float32),
        "pk32": (np.arange(128)[:, None] % 32 == np.arange(32)[None, :]).astype(np.float32),
    }

    # ---------------- build the SPMD program ----------------
    nc = bacc.Bacc("TRN2", target_bir_lowering=False, debug=False, num_devices=NC)

    def din(name, shape, dtype=f32):
        return nc.declare_dram_parameter(name, list(shape), dtype, isOutput=False)

    t_vsh = din("vsh", [B, NPC, C])
    t_fsh = din("fsh", [B, FPC, C])
    t_w0f = din("w0f", [128, 128]); t_w0x = din("w0x", [128, 128])
    t_w1f = din("w1f", [128, 128]); t_w1x = din("w1x", [128, 128])
    t_bn0 = din("bn0v", [4, 128]); t_bn1 = din("bn1v", [4, 128])
    t_fb0 = din("fb0", [1, 128]); t_fb1 = din("fb1", [1, 128])
    t_iota = din("iota", [128, 128]); t_id = din("ident", [128, 128])
    t_ones = din("onesc", [128, 1]); t_onesr = din("onesr", [1, 128])
    t_pk = din("pk32", [128, 32])
    p1idx = [din(f"p1idx{w}", [128, ncols1[w] * 8], mybir.dt.int16) for w in range(W1)]
    p1rel = [din(f"p1rel{w}", [128, max(nsl1[w], 1)]) for w in range(W1)]
    p1val = [din(f"p1val{w}", [128, max(nsl1[w], 1)]) for w in range(W1)]
    p3idx = [din(f"p3idx{w}", [128, ncols2[w] * 8], mybir.dt.int16) for w in range(W2)]
    p3rel = [din(f"p3rel{w}", [128, max(nsl2[w], 1)]) for w in range(W2)]
    p3val = [din(f"p3val{w}", [128, max(nsl2[w], 1)]) for w in range(W2)]

    t_vout = nc.declare_dram_parameter("vout", [B, NPC, C], f32, isOutput=True)
    t_fout = nc.declare_dram_parameter("fout", [B, FPC, C], f32, isOutput=True)

    t1sh = nc.dram_tensor("t1sh", [NPC * 4, 128], f32)
    t1full = nc.dram_tensor("t1full", [T1_ROWS, 128], f32)
    t2sh = nc.dram_tensor("t2sh", [FPC * 4, 128], f32)
    t2full = nc.dram_tensor("t2full", [T2_ROWS, 128], f32)
    m1r = nc.dram_tensor("m1r", [4, 32, 4, FPC], f32)
    m2r = nc.dram_tensor("m2r", [4, 32, 4, NPC], f32)
    st0_loc = nc.dram_tensor("st0_loc", [4, 128], f32)
    st0_glob = nc.dram_tensor("st0_glob", [4, 128], f32)
    st1_loc = nc.dram_tensor("st1_loc", [4, 128], f32)
    st1_glob = nc.dram_tensor("st1_glob", [4, 128], f32)

    import os
    PHMAX = int(os.environ.get("KPHASES", "9"))
    KSEG = int(os.environ.get("KSEG", "3"))
    COUNT0 = float(B * F_)
    COUNT1 = float(B * N)
    RG = [list(range(NC))]

    with tile.TileContext(nc) as tc:
        cpool = tc.alloc_tile_pool(name="const", bufs=1)
        iota_t = cpool.tile([128, 128], f32)
        nc.sync.dma_start(out=iota_t[:], in_=t_iota[:])
        id_t = cpool.tile([128, 128], f32)
        nc.sync.dma_start(out=id_t[:], in_=t_id[:])
        ones_t = cpool.tile([128, 1], f32)
        nc.sync.dma_start(out=ones_t[:], in_=t_ones[:])
        onesr_t = cpool.tile([1, 128], f32)
        nc.sync.dma_start(out=onesr_t[:], in_=t_onesr[:])

        def elu(pool, src_ap, Fdim, tag):
            """returns (x_tile, elu_tile) both [128, Fdim]"""
            xt = pool.tile([128, Fdim], f32, tag=tag + "x")
            nc.sync.dma_start(out=xt[:], in_=src_ap)
            mt = pool.tile([128, Fdim], f32, tag=tag + "m")
            nc.vector.tensor_scalar(out=mt[:], in0=xt[:], scalar1=0.0,
                                    scalar2=None, op0=OP.min)
            nc.scalar.activation(out=mt[:], in_=mt[:], func=AF.Exp)
            nc.vector.tensor_scalar(out=mt[:], in0=mt[:], scalar1=1.0,
                                    scalar2=None, op0=OP.subtract)
            ot = pool.tile([128, Fdim], f32, tag=tag + "o")
            nc.vector.tensor_tensor(out=ot[:], in0=xt[:], in1=mt[:], op=OP.max)
            return xt, ot

        # ================= phase 0: T1 build + input stats =================
        with tc.tile_pool(name="p0", bufs=3) as pool, \
             tc.tile_pool(name="p0ps", bufs=1, space="PSUM") as pps:
            xs_ps = pps.tile([128, 1], f32, space="PSUM", tag="xs")
            xq_ps = pps.tile([128, 1], f32, space="PSUM", tag="xq")
            fs_ps = pps.tile([128, 1], f32, space="PSUM", tag="fs")
            fq_ps = pps.tile([128, 1], f32, space="PSUM", tag="fq")
            n_nch = NPC // 128
            step = 0
            last_step = n_nch * B - 1
            for ci in range(n_nch):
                n0 = ci * 128
                blk = pool.tile([128, 512], f32, tag="t1blk")
                for b in range(B):
                    _, et = elu(pool, t_vsh[b, n0:n0 + 128, :], 128, "v")
                    # scatter into (s, b, k) positions of blk
                    nc.scalar.activation(
                        out=blk[:].rearrange("p (s bb k) -> p bb s k", s=4, bb=4)[:, b],
                        in_=et[:].rearrange("p (s k) -> p s k", s=4),
                        func=AF.Copy)
                    sq = pool.tile([128, 128], f32, tag="sqx")
                    nc.scalar.activation(out=sq[:], in_=et[:], func=AF.Square)
                    nc.tensor.matmul(out=xs_ps[:], lhsT=et[:], rhs=ones_t[:],
                                     start=(step == 0), stop=(step == last_step))
                    nc.tensor.matmul(out=xq_ps[:], lhsT=sq[:], rhs=ones_t[:],
                                     start=(step == 0), stop=(step == last_step))
                    step += 1
                nc.sync.dma_start(
                    out=t1sh[n0 * 4:(n0 + 128) * 4, :].rearrange("(n s) c -> n s c", s=4),
                    in_=blk[:].rearrange("p (s c) -> p s c", s=4))
            # f stats
            n_fch = FPC // 128
            step = 0
            last_step = n_fch * B - 1
            for ci in range(n_fch):
                f0 = ci * 128
                for b in range(B):
                    _, et = elu(pool, t_fsh[b, f0:f0 + 128, :], 128, "f")
                    sq = pool.tile([128, 128], f32, tag="sqf")
                    nc.scalar.activation(out=sq[:], in_=et[:], func=AF.Square)
                    nc.tensor.matmul(out=fs_ps[:], lhsT=et[:], rhs=ones_t[:],
                                     start=(step == 0), stop=(step == last_step))
                    nc.tensor.matmul(out=fq_ps[:], lhsT=sq[:], rhs=ones_t[:],
                                     start=(step == 0), stop=(step == last_step))
                    step += 1
            st = pool.tile([128, 4], f32, tag="stev")
            nc.scalar.activation(out=st[:, 0:1], in_=fs_ps[:], func=AF.Copy)
            nc.scalar.activation(out=st[:, 1:2], in_=fq_ps[:], func=AF.Copy)
            nc.scalar.activation(out=st[:, 2:3], in_=xs_ps[:], func=AF.Copy)
            nc.scalar.activation(out=st[:, 3:4], in_=xq_ps[:], func=AF.Copy)
            nc.sync.dma_start(out=st0_loc[0, :].rearrange("(p o) -> p o", o=1), in_=st[:, 0:1])
            nc.sync.dma_start(out=st0_loc[1, :].rearrange("(p o) -> p o", o=1), in_=st[:, 1:2])
            nc.sync.dma_start(out=st1_loc[0, :].rearrange("(p o) -> p o", o=1), in_=st[:, 2:3])
            nc.sync.dma_start(out=st1_loc[1, :].rearrange("(p o) -> p o", o=1), in_=st[:, 3:4])

        if PHMAX >= 2:
            nc.gpsimd.collective_compute("AllGather", OP.bypass, replica_groups=RG,
                                         ins=[t1sh[:]], outs=[t1full[:]])

        # ================= segment-sum phases =================
        def seg_phase(nblocks, nwin, chs, idxs_d, rel_d, val_d, slots, nsl,
                      table, out_r, out_fdim, pfx):
            with tc.tile_pool(name=pfx + "meta", bufs=1) as mpool, \
                 tc.tile_pool(name=pfx + "sb", bufs=4) as pool, \
                 tc.tile_pool(name=pfx + "g", bufs=2) as gpool, \
                 tc.tile_pool(name=pfx + "ps", bufs=4, space="PSUM") as pps:
                idx_ts, rel_ts, val_ts = [], [], []
                for w in range(nwin):
                    it = mpool.tile([128, idxs_d[w].shape[1]], mybir.dt.int16,
                                    tag=f"mi{w}")
                    nc.sync.dma_start(out=it[:], in_=idxs_d[w][:])
                    idx_ts.append(it)
                    rt = mpool.tile([128, rel_d[w].shape[1]], f32, tag=f"mr{w}")
                    nc.sync.dma_start(out=rt[:], in_=rel_d[w][:])
                    rel_ts.append(rt)
                    vt = mpool.tile([128, val_d[w].shape[1]], f32, tag=f"mv{w}")
                    nc.sync.dma_start(out=vt[:], in_=val_d[w][:])
                    val_ts.append(vt)
                cur_chunk = [None] * nwin
                cur_ci = [-1] * nwin
                gslot = [0] * nwin

                def ensure_chunk(w, ci):
                    if cur_ci[w] == ci:
                        return
                    g = gpool.tile([128, chs, 128], f32, tag=f"g{w}")
                    if KSEG != 2:
                        nc.gpsimd.dma_gather(
                            out_ap=g[:], in_ap=table[w * WIN:, :],
                            idxs_ap=idx_ts[w][:, ci * chs * 8:(ci + 1) * chs * 8],
                            num_idxs=chs * 128, num_idxs_reg=chs * 128, elem_size=128,
                            single_packet=False)
                    else:
                        nc.vector.memset(g[:, 0, :], 0.0)
                    cur_chunk[w] = g
                    cur_ci[w] = ci

                for b in range(nblocks):
                    if KSEG == 1:
                        for w in set(slots[b]):
                            gs = gslot[w]
                            ensure_chunk(w, gs // chs)
                        for w in slots[b]:
                            gslot[w] += 1
                        continue
                    ps = pps.tile([128, 128], f32, space="PSUM", tag="seg")
                    sl = slots[b]
                    for i, w in enumerate(sl):
                        gs = gslot[w]
                        ensure_chunk(w, gs // chs)
                        j = gs % chs
                        sel = pool.tile([128, 128], f32, tag="sel")
                        nc.vector.tensor_scalar(
                            out=sel[:], in0=iota_t[:],
                            scalar1=rel_ts[w][:, gs:gs + 1],
                            scalar2=val_ts[w][:, gs:gs + 1],
                            op0=OP.is_equal, op1=OP.mult)
                        nc.tensor.matmul(out=ps[:], lhsT=cur_chunk[w][:, j, :],
                                         rhs=sel[:], start=(i == 0),
                                         stop=(i == len(sl) - 1))
                        gslot[w] += 1
                    mt = pool.tile([128, 128], f32, tag="mev")
                    nc.scalar.activation(
                        out=mt[:].rearrange("p (s fl) -> p s fl", s=4),
                        in_=ps[:].rearrange("p (fl s) -> p s fl", s=4),
                        func=AF.Copy)
                    f0 = b * 32
                    nc.sync.dma_start(
                        out=out_r[:, :, :, f0:f0 + 32],
                        in_=mt[:].rearrange("p (s fl) -> p s fl", s=4))

        if PHMAX >= 3:
            seg_phase(NB1, W1, CH1, p1idx, p1rel, p1val, slots1, nsl1,
                      t1full, m1r, FPC, "s1")

        # ---- stats over m1r + allreduce + W0' build ----
        def m_stats(src, fdim, st_loc, st_glob):
            with tc.tile_pool(name="mst", bufs=3) as pool, \
                 tc.tile_pool(name="mstps", bufs=1, space="PSUM") as pps:
                acc_s = pool.tile([128, 4], f32, tag="accs")
                acc_q = pool.tile([128, 4], f32, tag="accq")
                nc.vector.memset(acc_s[:], 0.0)
                nc.vector.memset(acc_q[:], 0.0)
                nch = fdim // 512
                rem = fdim - nch * 512
                spans = [(i * 512, 512) for i in range(nch)]
                if rem:
                    spans.append((nch * 512, rem))
                for (f0, ln) in spans:
                    xt = pool.tile([128, 4, 512], f32, tag="mstx")
                    nc.sync.dma_start(out=xt[:, :, :ln], in_=src[:, :, :, f0:f0 + ln])
                    sq = pool.tile([128, 4, 512], f32, tag="mstq")
                    nc.scalar.activation(out=sq[:, :, :ln], in_=xt[:, :, :ln],
                                         func=AF.Square)
                    for s in range(4):
                        t1 = pool.tile([128, 1], f32, tag="mr1")
                        nc.vector.reduce_sum(out=t1[:], in_=xt[:, s, :ln], axis=mybir.AxisListType.X)
                        nc.vector.tensor_tensor(out=acc_s[:, s:s + 1],
                                                in0=acc_s[:, s:s + 1], in1=t1[:],
                                                op=OP.add)
                        t2 = pool.tile([128, 1], f32, tag="mr2")
                        nc.vector.reduce_sum(out=t2[:], in_=sq[:, s, :ln], axis=mybir.AxisListType.X)
                        nc.vector.tensor_tensor(out=acc_q[:, s:s + 1],
                                                in0=acc_q[:, s:s + 1], in1=t2[:],
                                                op=OP.add)
                # fold b: out[s, k] = sum_b acc[(b,k), s] via matmul with Pk
                pk = pool.tile([128, 32], f32, tag="pk")
                # build Pk = (iota32 == k_index): k index per partition = p % 32
                # use iota columns 0..31 compared to (p%32): precompute on host? use
                # iota_t[:, :32] == pmod tile: simplest: DMA from host const? reuse:
                # Pk[p, j] = (p % 32 == j): tensor_scalar(is_equal) with scalar AP =
                # pmod values: pmod[p] = p % 32 -> supply via iota trick:
                # iota_t[:, :1] is 0 for all p. Instead load from host const.
                nc.sync.dma_start(out=pk[:], in_=t_pk[:])
                fold_s = pps.tile([4, 32], f32, space="PSUM", tag="folds")
                fold_q = pps.tile([4, 32], f32, space="PSUM", tag="foldq")
                nc.tensor.matmul(out=fold_s[:], lhsT=acc_s[:], rhs=pk[:],
                                 start=True, stop=True)
                nc.tensor.matmul(out=fold_q[:], lhsT=acc_q[:], rhs=pk[:],
                                 start=True, stop=True)
                ev = pool.tile([4, 64], f32, tag="mfev")
                nc.scalar.activation(out=ev[:, :32], in_=fold_s[:], func=AF.Copy)
                nc.scalar.activation(out=ev[:, 32:], in_=fold_q[:], func=AF.Copy)
                nc.sync.dma_start(out=st_loc[2, :].rearrange("(s k) -> s k", s=4),
                                  in_=ev[:, :32])
                nc.sync.dma_start(out=st_loc[3, :].rearrange("(s k) -> s k", s=4),
                                  in_=ev[:, 32:])
            nc.gpsimd.collective_compute("AllReduce", OP.add, replica_groups=RG,
                                         ins=[st_loc[:]], outs=[st_glob[:]])

        def bn_fold(st_glob, t_bnv, t_w_f, t_w_x, t_fb, count, wpool):
            """returns (wf_scaled, wx_scaled, bias_row) tiles in wpool"""
            p = wpool
            sum_f = p.tile([128, 1], f32, tag="bsf")
            nc.sync.dma_start(out=sum_f[:], in_=st_glob[0, :].rearrange("(p o) -> p o", o=1))
            sq_f = p.tile([128, 1], f32, tag="bqf")
            nc.sync.dma_start(out=sq_f[:], in_=st_glob[1, :].rearrange("(p o) -> p o", o=1))
            sum_x = p.tile([128, 1], f32, tag="bsx")
            nc.sync.dma_start(out=sum_x[:],
                              in_=st_glob[2, :].rearrange("(s k o) -> k s o", s=4, o=1))
            sq_x = p.tile([128, 1], f32, tag="bqx")
            nc.sync.dma_start(out=sq_x[:],
                              in_=st_glob[3, :].rearrange("(s k o) -> k s o", s=4, o=1))
            g_f = p.tile([128, 1], f32, tag="bgf")
            nc.sync.dma_start(out=g_f[:], in_=t_bnv[0, :].rearrange("(p o) -> p o", o=1))
            be_f = p.tile([128, 1], f32, tag="bbf")
            nc.sync.dma_start(out=be_f[:], in_=t_bnv[1, :].rearrange("(p o) -> p o", o=1))
            g_x = p.tile([128, 1], f32, tag="bgx")
            nc.sync.dma_start(out=g_x[:], in_=t_bnv[2, :].rearrange("(p o) -> p o", o=1))
            be_x = p.tile([128, 1], f32, tag="bbx")
            nc.sync.dma_start(out=be_x[:], in_=t_bnv[3, :].rearrange("(p o) -> p o", o=1))

            outs = []
            for (sm, sq, ga, be, t_w, tg) in ((sum_f, sq_f, g_f, be_f, t_w_f, "f"),
                                              (sum_x, sq_x, g_x, be_x, t_w_x, "x")):
                mu = p.tile([128, 1], f32, tag="bmu" + tg)
                nc.vector.tensor_scalar(out=mu[:], in0=sm[:], scalar1=1.0 / count,
                                        scalar2=None, op0=OP.mult)
                var = p.tile([128, 1], f32, tag="bvar" + tg)
                nc.vector.tensor_scalar(out=var[:], in0=sq[:], scalar1=1.0 / count,
                                        scalar2=None, op0=OP.mult)
                mu2 = p.tile([128, 1], f32, tag="bmu2" + tg)
                nc.vector.tensor_tensor(out=mu2[:], in0=mu[:], in1=mu[:], op=OP.mult)
                nc.vector.tensor_tensor(out=var[:], in0=var[:], in1=mu2[:], op=OP.subtract)
                nc.vector.tensor_scalar(out=var[:], in0=var[:], scalar1=EPS,
                                        scalar2=None, op0=OP.add)
                sd = p.tile([128, 1], f32, tag="bsd" + tg)
                nc.scalar.activation(out=sd[:], in_=var[:], func=AF.Sqrt)
                rs = p.tile([128, 1], f32, tag="brs" + tg)
                nc.vector.reciprocal(out=rs[:], in_=sd[:])
                A = p.tile([128, 1], f32, tag="bA" + tg)
                nc.vector.tensor_tensor(out=A[:], in0=rs[:], in1=ga[:], op=OP.mult)
                bc = p.tile([128, 1], f32, tag="bbc" + tg)
                nc.vector.tensor_tensor(out=bc[:], in0=mu[:], in1=A[:], op=OP.mult)
                nc.vector.tensor_tensor(out=bc[:], in0=be[:], in1=bc[:], op=OP.subtract)
                wt = p.tile([128, 128], f32, tag="bwt" + tg)
                nc.sync.dma_start(out=wt[:], in_=t_w[:])
                ws = p.tile([128, 128], f32, tag="bws" + tg)
                nc.scalar.activation(out=ws[:], in_=wt[:], func=AF.Copy, scale=A[:, :1])
                outs.append((wt, ws, bc))
            (wtf, wsf, bcf), (wtx, wsx, bcx) = outs
            with tc.tile_pool(name="bnps", bufs=1, space="PSUM") as bps:
                bp = bps.tile([1, 128], f32, space="PSUM", tag="bp")
                nc.tensor.matmul(out=bp[:], lhsT=bcf[:], rhs=wtf[:], start=True, stop=False)
                nc.tensor.matmul(out=bp[:], lhsT=bcx[:], rhs=wtx[:], start=False, stop=True)
                fb = p.tile([1, 128], f32, tag="bfb")
                nc.sync.dma_start(out=fb[:], in_=t_fb[:])
                brow = p.tile([1, 128], f32, tag="bbrow")
                nc.vector.tensor_tensor(out=brow[:], in0=bp[:], in1=fb[:], op=OP.add)
            return wsf, wsx, brow

        if PHMAX >= 4:
            m_stats(m1r, FPC, st0_loc, st0_glob)
        if PHMAX >= 5:
            wpool0 = tc.alloc_tile_pool(name="w0pool", bufs=1)
            w0fp, w0xp, b0row = bn_fold(st0_glob, t_bn0, t_w0f, t_w0x, t_fb0, COUNT0, wpool0)

        # ================= phase 2: BN0-linear + f_out + T2 =================
        def lin_phase(npart_chunks, t_in_sh, m_r, wfp, wxp, brow, t_out,
                      t2blk_dst, residual, pfx):
            with tc.tile_pool(name=pfx + "sb", bufs=4) as pool, \
                 tc.tile_pool(name=pfx + "ps", bufs=3, space="PSUM") as pps, \
                 tc.tile_pool(name=pfx + "tps", bufs=2, space="PSUM") as tpps:
                for ci in range(npart_chunks):
                    f0 = ci * 128
                    if t2blk_dst is not None:
                        blk = pool.tile([128, 512], f32, tag="t2blk", name="t2blk")
                    else:
                        blk = None
                    for b in range(B):
                        xt, et = elu(pool, t_in_sh[b, f0:f0 + 128, :], 128, pfx + "e")
                        tp = tpps.tile([128, 128], f32, space="PSUM", tag="tp")
                        nc.tensor.transpose(out=tp[:], in_=et[:], identity=id_t[:])
                        lf = pool.tile([128, 128], f32, tag="lf")
                        nc.scalar.activation(out=lf[:], in_=tp[:], func=AF.Copy)
                        lx = pool.tile([128, 128], f32, tag="lx")
                        nc.sync.dma_start(out=lx[:], in_=m_r[b, :, :, f0:f0 + 128])
                        ps_o = pps.tile([128, 128], f32, space="PSUM", tag="po")
                        nc.tensor.matmul(out=ps_o[:], lhsT=lf[:], rhs=wfp[:],
                                         start=True, stop=False)
                        nc.tensor.matmul(out=ps_o[:], lhsT=lx[:], rhs=wxp[:],
                                         start=False, stop=False)
                        nc.tensor.matmul(out=ps_o[:], lhsT=onesr_t[:], rhs=brow[:],
                                         start=False, stop=True)
                        if residual:
                            ot = pool.tile([128, 128], f32, tag="ot")
                            nc.vector.tensor_tensor(out=ot[:], in0=ps_o[:], in1=xt[:],
                                                    op=OP.add)
                            nc.sync.dma_start(out=t_out[b, f0:f0 + 128, :], in_=ot[:])
                        else:
                            ot = pool.tile([128, 128], f32, tag="ot")
                            nc.scalar.activation(out=ot[:], in_=ps_o[:], func=AF.Copy)
                            nc.sync.dma_start(out=t_out[b, f0:f0 + 128, :], in_=ot[:])
                        if blk is not None:
                            m2 = pool.tile([128, 128], f32, tag="m2")
                            nc.vector.tensor_scalar(out=m2[:], in0=ps_o[:], scalar1=0.0,
                                                    scalar2=None, op0=OP.min)
                            nc.scalar.activation(out=m2[:], in_=m2[:], func=AF.Exp)
                            nc.vector.tensor_scalar(out=m2[:], in0=m2[:], scalar1=1.0,
                                                    scalar2=None, op0=OP.subtract)
                            nc.vector.tensor_tensor(
                                out=blk[:].rearrange("p (s bb k) -> p bb s k", s=4, bb=4)[:, b],
                                in0=ps_o[:].rearrange("p (s k) -> p s k", s=4),
                                in1=m2[:].rearrange("p (s k) -> p s k", s=4),
                                op=OP.max)
                    if blk is not None:
                        nc.sync.dma_start(
                            out=t2blk_dst[f0 * 4:(f0 + 128) * 4, :]
                            .rearrange("(n s) c -> n s c", s=4),
                            in_=blk[:].rearrange("p (s c) -> p s c", s=4))

        if PHMAX >= 5:
            lin_phase(FPC // 128, t_fsh, m1r, w0fp, w0xp, b0row, t_fout, t2sh, False, "l2")
            wpool0.release()

        if PHMAX >= 6:
            nc.gpsimd.collective_compute("AllGather", OP.bypass, replica_groups=RG,
                                         ins=[t2sh[:]], outs=[t2full[:]])

        # ================= phase 3: spbmm2 (DiA) =================
        if PHMAX >= 7:
            seg_phase(NB2, W2, CH2, p3idx, p3rel, p3val, slots2, nsl2,
                      t2full, m2r, NPC, "s3")

        if PHMAX >= 8:
            m_stats(m2r, NPC, st1_loc, st1_glob)
        if PHMAX >= 9:
            wpool1 = tc.alloc_tile_pool(name="w1pool", bufs=1)
            w1fp, w1xp, b1row = bn_fold(st1_glob, t_bn1, t_w1f, t_w1x, t_fb1, COUNT1, wpool1)
            lin_phase(NPC // 128, t_vsh, m2r, w1fp, w1xp, b1row, t_vout, None, True, "l4")
            wpool1.release()
        cpool.release()

    nc.compile()

    in_maps = []
    for c in range(NC):
        m = {
            "vsh": vpad[:, c * NPC:(c + 1) * NPC, :].copy(),
            "fsh": fpad[:, c * FPC:(c + 1) * FPC, :].copy(),
        }
        m.update(consts)
        m.update(meta[c])
        in_maps.append(m)

    trace = bool(int(os.environ.get("KTRACE", "0")))
    res = run_bass_kernel_spmd(nc, in_maps, core_ids=list(range(NC)), trace=trace)
    kernel.last_exec_time_ns = getattr(res, "exec_time_ns", None)

    vout = np.concatenate([res.results[c]["vout"] for c in range(NC)], axis=1)[:, :N]
    fout = np.concatenate([res.results[c]["fout"] for c in range(NC)], axis=1)[:, :F_]
    return vout, fout


# revision 11
# speedup vs baseline: 3.4797x; 3.4797x over previous
import sys

for p in ('/opt/trn_rl_repo', '/root/problem'):
    if p not in sys.path:
        sys.path.insert(0, p)

import numpy as np

# ---- problem constants (hardcoded per contract) ----
B, N, F_, C = 4, 30000, 60000, 128
NNZ = 720000
EPS = 1e-5
NC = 8
NPC = 3840                  # nodes per core (8*3840 = 30720 >= 30000), /128
FPC = 7552                  # faces per core (8*7552 = 60416 >= 60000), /128
NR1 = FPC * 4               # dest rows per core, phase 1
NR2 = NPC * 4               # dest rows per core, phase 3
NB1 = NR1 // 128            # 236
NB2 = NR2 // 128            # 120
T1_ROWS = NC * NPC * 4      # 122880
T2_ROWS = NC * FPC * 4      # 241664
W1 = 4
W2 = 8
WIN = 32768
CH1 = 16                    # slots per gather chunk (2048 entries), phase 1
CH2 = 16                    # slots per gather chunk (2048 entries), phase 3


def _schedule(dst_core, dst_rel, src_row, vals, nblocks, nwin):
    blk = dst_rel // 128
    rel = (dst_rel % 128).astype(np.float32)
    win = src_row // WIN
    idx16 = (src_row - win * WIN).astype(np.int16)

    counts = np.zeros((NC, nblocks, nwin), dtype=np.int64)
    np.add.at(counts, (dst_core, blk, win), 1)
    need = (counts.max(axis=0) + 127) // 128
    need = np.maximum(need, 1)
    slots = [[w for w in range(nwin) for _ in range(int(need[b, w]))]
             for b in range(nblocks)]
    n_slots_w = [int(need[:, w].sum()) for w in range(nwin)]
    base_w = []
    for w in range(nwin):
        base = np.zeros(nblocks + 1, dtype=np.int64)
        pos = 0
        for b in range(nblocks):
            base[b] = pos
            pos += int(need[b, w]) * 128
        base[nblocks] = pos
        base_w.append(base)

    per_core = []
    for c in range(NC):
        m = dst_core == c
        cb, cw = blk[m], win[m]
        order = np.lexsort((cw, cb))
        cb, cw = cb[order], cw[order]
        cr, cv, ci = rel[m][order], vals[m][order], idx16[m][order]
        streams = []
        for w in range(nwin):
            ns = n_slots_w[w]
            si = np.zeros(ns * 128, dtype=np.int16)
            sr = np.zeros(ns * 128, dtype=np.float32)
            sv = np.zeros(ns * 128, dtype=np.float32)
            sel = cw == w
            eb = cb[sel]
            off = np.zeros(len(eb), dtype=np.int64)
            if len(eb):
                change = np.flatnonzero(np.diff(eb) != 0) + 1
                starts = np.concatenate(([0], change))
                lens = np.diff(np.concatenate((starts, [len(eb)])))
                off = np.arange(len(eb)) - np.repeat(starts, lens)
            dst = base_w[w][eb] + off
            si[dst] = ci[sel]
            sr[dst] = cr[sel]
            sv[dst] = cv[sel]
            streams.append((si, sr, sv))
        per_core.append(streams)
    return slots, n_slots_w, per_core


def _wrap16(si, n_slots, chunk_slots):
    ns_pad = max(((n_slots + chunk_slots - 1) // chunk_slots) * chunk_slots, chunk_slots)
    full = np.zeros(ns_pad * 128, dtype=np.int16)
    full[:len(si)] = si
    cols = ns_pad * 8
    w = np.zeros((128, cols), dtype=np.int16)
    idx = np.arange(ns_pad * 128)
    w[idx % 16, idx // 16] = full
    for g in range(1, 8):
        w[g * 16:(g + 1) * 16] = w[:16]
    return w, ns_pad


def _slotpack(arr, n_slots):
    out = np.zeros((128, max(n_slots, 1)), dtype=np.float32)
    if n_slots:
        out[:, :n_slots] = arr.reshape(n_slots, 128).T
    return out


def kernel(Di_rows, Di_cols, Di_vals, DiA_rows, DiA_cols, DiA_vals, v, f,
           bn0_gamma, bn0_beta, fc0_w, fc0_b, bn1_gamma, bn1_beta, fc1_w, fc1_b):
    import concourse.bass as bass
    import concourse.bacc as bacc
    import concourse.tile as tile
    from concourse import mybir
    from concourse.bass_utils import run_bass_kernel_spmd

    f32 = mybir.dt.float32
    bf16 = mybir.dt.bfloat16
    AF = mybir.ActivationFunctionType
    OP = mybir.AluOpType

    Di_rows = np.asarray(Di_rows); Di_cols = np.asarray(Di_cols)
    Di_vals = np.asarray(Di_vals, dtype=np.float32)
    DiA_rows = np.asarray(DiA_rows); DiA_cols = np.asarray(DiA_cols)
    DiA_vals = np.asarray(DiA_vals, dtype=np.float32)
    v = np.asarray(v, dtype=np.float32); f = np.asarray(f, dtype=np.float32)
    bn0_gamma = np.asarray(bn0_gamma, dtype=np.float32)
    bn0_beta = np.asarray(bn0_beta, dtype=np.float32)
    fc0_w = np.asarray(fc0_w, dtype=np.float32)
    fc0_b = np.asarray(fc0_b, dtype=np.float32)
    bn1_gamma = np.asarray(bn1_gamma, dtype=np.float32)
    bn1_beta = np.asarray(bn1_beta, dtype=np.float32)
    fc1_w = np.asarray(fc1_w, dtype=np.float32)
    fc1_b = np.asarray(fc1_b, dtype=np.float32)

    # ---------------- host-side index preprocessing ----------------
    r = Di_rows.astype(np.int64)
    fc = r // 4
    d1_core = fc // FPC
    d1_rel = 4 * (fc - d1_core * FPC) + (r % 4)
    c4 = Di_cols.astype(np.int64)
    nd = c4 // 4
    scr = nd // NPC
    t1row = scr * (NPC * 4) + 4 * (nd - scr * NPC) + (c4 % 4)
    slots1, nsl1, pc1 = _schedule(d1_core, d1_rel, t1row, Di_vals, NB1, W1)

    r2 = DiA_rows.astype(np.int64)
    nd2 = r2 // 4
    d2_core = nd2 // NPC
    d2_rel = 4 * (nd2 - d2_core * NPC) + (r2 % 4)
    c42 = DiA_cols.astype(np.int64)
    fc2 = c42 // 4
    sc2 = fc2 // FPC
    t2row = sc2 * (FPC * 4) + 4 * (fc2 - sc2 * FPC) + (c42 % 4)
    slots2, nsl2, pc2 = _schedule(d2_core, d2_rel, t2row, DiA_vals, NB2, W2)

    meta = []
    ncols1 = [None] * W1
    ncols2 = [None] * W2
    for c in range(NC):
        m = {}
        for w in range(W1):
            si, sr, sv = pc1[c][w]
            wi, ns_pad = _wrap16(si, nsl1[w], CH1)
            ncols1[w] = ns_pad
            m[f"p1idx{w}"] = wi
            m[f"p1rel{w}"] = _slotpack(sr, nsl1[w])
            m[f"p1val{w}"] = _slotpack(sv, nsl1[w])
        for w in range(W2):
            si, sr, sv = pc2[c][w]
            wi, ns_pad = _wrap16(si, nsl2[w], CH2)
            ncols2[w] = ns_pad
            m[f"p3idx{w}"] = wi
            m[f"p3rel{w}"] = _slotpack(sr, nsl2[w])
            m[f"p3val{w}"] = _slotpack(sv, nsl2[w])
        meta.append(m)

    vpad = np.zeros((B, NC * NPC, C), dtype=np.float32)
    vpad[:, :N] = v
    fpad = np.zeros((B, NC * FPC, C), dtype=np.float32)
    fpad[:, :F_] = f

    kk = np.arange(32); ss = np.arange(4)
    perm_ks = (128 + 32 * ss[None, :] + kk[:, None]).reshape(-1)
    W0T = fc0_w.T
    W1T = fc1_w.T
    consts = {
        "w0f": W0T[:128].copy(), "w0x": W0T[perm_ks].copy(),
        "w1f": W1T[:128].copy(), "w1x": W1T[perm_ks].copy(),
        "bn0v": np.stack([bn0_gamma[:128], bn0_beta[:128],
                          bn0_gamma[perm_ks], bn0_beta[perm_ks]]),
        "bn1v": np.stack([bn1_gamma[:128], bn1_beta[:128],
                          bn1_gamma[perm_ks], bn1_beta[perm_ks]]),
        "fb0": fc0_b.reshape(1, 128).copy(), "fb1": fc1_b.reshape(1, 128).copy(),
        "iota": np.tile(np.arange(128, dtype=np.float32), (128, 1)),
        "ident": np.eye(128, dtype=np.float32),
        "onesc": np.ones((128, 1), dtype=np.float32),
        "onesr": np.ones((1, 128), dtype=np.float32),
        "pk32": (np.arange(128)[:, None] % 32 == np.arange(32)[None, :]).astype(np.float32),
    }

    # ---------------- build the SPMD program ----------------
    nc = bacc.Bacc("TRN2", target_bir_lowering=False, debug=False, num_devices=NC)

    def din(name, shape, dtype=f32):
        return nc.declare_dram_parameter(name, list(shape), dtype, isOutput=False)

    t_vsh = din("vsh", [B, NPC, C])
    t_fsh = din("fsh", [B, FPC, C])
    t_w0f = din("w0f", [128, 128]); t_w0x = din("w0x", [128, 128])
    t_w1f = din("w1f", [128, 128]); t_w1x = din("w1x", [128, 128])
    t_bn0 = din("bn0v", [4, 128]); t_bn1 = din("bn1v", [4, 128])
    t_fb0 = din("fb0", [1, 128]); t_fb1 = din("fb1", [1, 128])
    t_iota = din("iota", [128, 128]); t_id = din("ident", [128, 128])
    t_ones = din("onesc", [128, 1]); t_onesr = din("onesr", [1, 128])
    t_pk = din("pk32", [128, 32])
    p1idx = [din(f"p1idx{w}", [128, ncols1[w] * 8], mybir.dt.int16) for w in range(W1)]
    p1rel = [din(f"p1rel{w}", [128, max(nsl1[w], 1)]) for w in range(W1)]
    p1val = [din(f"p1val{w}", [128, max(nsl1[w], 1)]) for w in range(W1)]
    p3idx = [din(f"p3idx{w}", [128, ncols2[w] * 8], mybir.dt.int16) for w in range(W2)]
    p3rel = [din(f"p3rel{w}", [128, max(nsl2[w], 1)]) for w in range(W2)]
    p3val = [din(f"p3val{w}", [128, max(nsl2[w], 1)]) for w in range(W2)]

    t_vout = nc.declare_dram_parameter("vout", [B, NPC, C], f32, isOutput=True)
    t_fout = nc.declare_dram_parameter("fout", [B, FPC, C], f32, isOutput=True)

    t1sh = nc.dram_tensor("t1sh", [NPC * 4, 128], bf16)
    t1full = nc.dram_tensor("t1full", [T1_ROWS, 128], bf16)
    t2sh = nc.dram_tensor("t2sh", [FPC * 4, 128], bf16)
    t2full = nc.dram_tensor("t2full", [T2_ROWS, 128], bf16)
    m1r = nc.dram_tensor("m1r", [4, 32, 4, FPC], f32)
    m2r = nc.dram_tensor("m2r", [4, 32, 4, NPC], f32)
    st0_loc = nc.dram_tensor("st0_loc", [4, 128], f32)
    st0_glob = nc.dram_tensor("st0_glob", [4, 128], f32)
    st1_loc = nc.dram_tensor("st1_loc", [4, 128], f32)
    st1_glob = nc.dram_tensor("st1_glob", [4, 128], f32)

    import os
    PHMAX = int(os.environ.get("KPHASES", "9"))
    KSEG = int(os.environ.get("KSEG", "3"))
    COUNT0 = float(B * F_)
    COUNT1 = float(B * N)
    RG = [list(range(NC))]

    with tile.TileContext(nc) as tc:
        cpool = tc.alloc_tile_pool(name="const", bufs=1)
        iota_t = cpool.tile([128, 128], f32)
        nc.sync.dma_start(out=iota_t[:], in_=t_iota[:])
        id_t = cpool.tile([128, 128], f32)
        nc.sync.dma_start(out=id_t[:], in_=t_id[:])
        ones_t = cpool.tile([128, 1], f32)
        nc.sync.dma_start(out=ones_t[:], in_=t_ones[:])
        onesr_t = cpool.tile([1, 128], f32)
        nc.sync.dma_start(out=onesr_t[:], in_=t_onesr[:])

        def elu(pool, src_ap, Fdim, tag):
            """returns (x_tile, elu_tile) both [128, Fdim]"""
            xt = pool.tile([128, Fdim], f32, tag=tag + "x")
            nc.sync.dma_start(out=xt[:], in_=src_ap)
            mt = pool.tile([128, Fdim], f32, tag=tag + "m")
            nc.vector.tensor_scalar(out=mt[:], in0=xt[:], scalar1=0.0,
                                    scalar2=None, op0=OP.min)
            nc.scalar.activation(out=mt[:], in_=mt[:], func=AF.Exp)
            nc.vector.tensor_scalar(out=mt[:], in0=mt[:], scalar1=1.0,
                                    scalar2=None, op0=OP.subtract)
            ot = pool.tile([128, Fdim], f32, tag=tag + "o")
            nc.vector.tensor_tensor(out=ot[:], in0=xt[:], in1=mt[:], op=OP.max)
            return xt, ot

        # ================= phase 0: T1 build + input stats =================
        with tc.tile_pool(name="p0", bufs=3) as pool, \
             tc.tile_pool(name="p0ps", bufs=1, space="PSUM") as pps:
            xs_ps = pps.tile([128, 1], f32, space="PSUM", tag="xs")
            xq_ps = pps.tile([128, 1], f32, space="PSUM", tag="xq")
            fs_ps = pps.tile([128, 1], f32, space="PSUM", tag="fs")
            fq_ps = pps.tile([128, 1], f32, space="PSUM", tag="fq")
            n_nch = NPC // 128
            step = 0
            last_step = n_nch * B - 1
            for ci in range(n_nch):
                n0 = ci * 128
                blk = pool.tile([128, 512], bf16, tag="t1blk")
                for b in range(B):
                    _, et = elu(pool, t_vsh[b, n0:n0 + 128, :], 128, "v")
                    # scatter into (s, b, k) positions of blk
                    nc.scalar.activation(
                        out=blk[:].rearrange("p (s bb k) -> p bb s k", s=4, bb=4)[:, b],
                        in_=et[:].rearrange("p (s k) -> p s k", s=4),
                        func=AF.Copy)
                    sq = pool.tile([128, 128], f32, tag="sqx")
                    nc.scalar.activation(out=sq[:], in_=et[:], func=AF.Square)
                    nc.tensor.matmul(out=xs_ps[:], lhsT=et[:], rhs=ones_t[:],
                                     start=(step == 0), stop=(step == last_step))
                    nc.tensor.matmul(out=xq_ps[:], lhsT=sq[:], rhs=ones_t[:],
                                     start=(step == 0), stop=(step == last_step))
                    step += 1
                nc.sync.dma_start(
                    out=t1sh[n0 * 4:(n0 + 128) * 4, :].rearrange("(n s) c -> n s c", s=4),
                    in_=blk[:].rearrange("p (s c) -> p s c", s=4))
            # f stats
            n_fch = FPC // 128
            step = 0
            last_step = n_fch * B - 1
            for ci in range(n_fch):
                f0 = ci * 128
                for b in range(B):
                    _, et = elu(pool, t_fsh[b, f0:f0 + 128, :], 128, "f")
                    sq = pool.tile([128, 128], f32, tag="sqf")
                    nc.scalar.activation(out=sq[:], in_=et[:], func=AF.Square)
                    nc.tensor.matmul(out=fs_ps[:], lhsT=et[:], rhs=ones_t[:],
                                     start=(step == 0), stop=(step == last_step))
                    nc.tensor.matmul(out=fq_ps[:], lhsT=sq[:], rhs=ones_t[:],
                                     start=(step == 0), stop=(step == last_step))
                    step += 1
            st = pool.tile([128, 4], f32, tag="stev")
            nc.scalar.activation(out=st[:, 0:1], in_=fs_ps[:], func=AF.Copy)
            nc.scalar.activation(out=st[:, 1:2], in_=fq_ps[:], func=AF.Copy)
            nc.scalar.activation(out=st[:, 2:3], in_=xs_ps[:], func=AF.Copy)
            nc.scalar.activation(out=st[:, 3:4], in_=xq_ps[:], func=AF.Copy)
            nc.sync.dma_start(out=st0_loc[0, :].rearrange("(p o) -> p o", o=1), in_=st[:, 0:1])
            nc.sync.dma_start(out=st0_loc[1, :].rearrange("(p o) -> p o", o=1), in_=st[:, 1:2])
            nc.sync.dma_start(out=st1_loc[0, :].rearrange("(p o) -> p o", o=1), in_=st[:, 2:3])
            nc.sync.dma_start(out=st1_loc[1, :].rearrange("(p o) -> p o", o=1), in_=st[:, 3:4])

        if PHMAX >= 2:
            nc.gpsimd.collective_compute("AllGather", OP.bypass, replica_groups=RG,
                                         ins=[t1sh[:]], outs=[t1full[:]])

        # ================= segment-sum phases =================
        def seg_phase(nblocks, nwin, chs, idxs_d, rel_d, val_d, slots, nsl,
                      table, out_r, out_fdim, pfx):
            with tc.tile_pool(name=pfx + "meta", bufs=1) as mpool, \
                 tc.tile_pool(name=pfx + "sb", bufs=4) as pool, \
                 tc.tile_pool(name=pfx + "g", bufs=3) as gpool, \
                 tc.tile_pool(name=pfx + "ps", bufs=4, space="PSUM") as pps:
                idx_ts, rel_ts, val_ts = [], [], []
                for w in range(nwin):
                    it = mpool.tile([128, idxs_d[w].shape[1]], mybir.dt.int16,
                                    tag=f"mi{w}")
                    nc.sync.dma_start(out=it[:], in_=idxs_d[w][:])
                    idx_ts.append(it)
                    rt = mpool.tile([128, rel_d[w].shape[1]], f32, tag=f"mr{w}")
                    nc.sync.dma_start(out=rt[:], in_=rel_d[w][:])
                    rel_ts.append(rt)
                    vt = mpool.tile([128, val_d[w].shape[1]], f32, tag=f"mv{w}")
                    nc.sync.dma_start(out=vt[:], in_=val_d[w][:])
                    val_ts.append(vt)
                cur_chunk = [None] * nwin
                cur_ci = [-1] * nwin
                gslot = [0] * nwin

                def ensure_chunk(w, ci):
                    if cur_ci[w] == ci:
                        return
                    g = gpool.tile([128, chs, 128], bf16, tag=f"g{w}")
                    if KSEG != 2:
                        nc.gpsimd.dma_gather(
                            out_ap=g[:], in_ap=table[w * WIN:, :],
                            idxs_ap=idx_ts[w][:, ci * chs * 8:(ci + 1) * chs * 8],
                            num_idxs=chs * 128, num_idxs_reg=chs * 128, elem_size=128,
                            single_packet=False)
                    else:
                        nc.vector.memset(g[:, 0, :], 0.0)
                    cur_chunk[w] = g
                    cur_ci[w] = ci

                for b in range(nblocks):
                    if KSEG == 1:
                        for w in set(slots[b]):
                            gs = gslot[w]
                            ensure_chunk(w, gs // chs)
                        for w in slots[b]:
                            gslot[w] += 1
                        continue
                    ps = pps.tile([128, 128], f32, space="PSUM", tag="seg")
                    sl = slots[b]
                    for i, w in enumerate(sl):
                        gs = gslot[w]
                        ensure_chunk(w, gs // chs)
                        j = gs % chs
                        sel = pool.tile([128, 128], bf16, tag="sel")
                        nc.vector.tensor_scalar(
                            out=sel[:], in0=iota_t[:],
                            scalar1=rel_ts[w][:, gs:gs + 1],
                            scalar2=val_ts[w][:, gs:gs + 1],
                            op0=OP.is_equal, op1=OP.mult)
                        nc.tensor.matmul(out=ps[:], lhsT=cur_chunk[w][:, j, :],
                                         rhs=sel[:], start=(i == 0),
                                         stop=(i == len(sl) - 1))
                        gslot[w] += 1
                    mt = pool.tile([128, 128], f32, tag="mev")
                    nc.scalar.activation(
                        out=mt[:].rearrange("p (s fl) -> p s fl", s=4),
                        in_=ps[:].rearrange("p (fl s) -> p s fl", s=4),
                        func=AF.Copy)
                    f0 = b * 32
                    nc.sync.dma_start(
                        out=out_r[:, :, :, f0:f0 + 32],
                        in_=mt[:].rearrange("p (s fl) -> p s fl", s=4))

        if PHMAX >= 3:
            seg_phase(NB1, W1, CH1, p1idx, p1rel, p1val, slots1, nsl1,
                      t1full, m1r, FPC, "s1")

        # ---- stats over m1r + allreduce + W0' build ----
        def m_stats(src, fdim, st_loc, st_glob):
            with tc.tile_pool(name="mst", bufs=3) as pool, \
                 tc.tile_pool(name="mstps", bufs=1, space="PSUM") as pps:
                acc_s = pool.tile([128, 4], f32, tag="accs")
                acc_q = pool.tile([128, 4], f32, tag="accq")
                nc.vector.memset(acc_s[:], 0.0)
                nc.vector.memset(acc_q[:], 0.0)
                nch = fdim // 512
                rem = fdim - nch * 512
                spans = [(i * 512, 512) for i in range(nch)]
                if rem:
                    spans.append((nch * 512, rem))
                for (f0, ln) in spans:
                    xt = pool.tile([128, 4, 512], f32, tag="mstx")
                    nc.sync.dma_start(out=xt[:, :, :ln], in_=src[:, :, :, f0:f0 + ln])
                    sq = pool.tile([128, 4, 512], f32, tag="mstq")
                    nc.scalar.activation(out=sq[:, :, :ln], in_=xt[:, :, :ln],
                                         func=AF.Square)
                    for s in range(4):
                        t1 = pool.tile([128, 1], f32, tag="mr1")
                        nc.vector.reduce_sum(out=t1[:], in_=xt[:, s, :ln], axis=mybir.AxisListType.X)
                        nc.vector.tensor_tensor(out=acc_s[:, s:s + 1],
                                                in0=acc_s[:, s:s + 1], in1=t1[:],
                                                op=OP.add)
                        t2 = pool.tile([128, 1], f32, tag="mr2")
                        nc.vector.reduce_sum(out=t2[:], in_=sq[:, s, :ln], axis=mybir.AxisListType.X)
                        nc.vector.tensor_tensor(out=acc_q[:, s:s + 1],
                                                in0=acc_q[:, s:s + 1], in1=t2[:],
                                                op=OP.add)
                # fold b: out[s, k] = sum_b acc[(b,k), s] via matmul with Pk
                pk = pool.tile([128, 32], f32, tag="pk")
                # build Pk = (iota32 == k_index): k index per partition = p % 32
                # use iota columns 0..31 compared to (p%32): precompute on host? use
                # iota_t[:, :32] == pmod tile: simplest: DMA from host const? reuse:
                # Pk[p, j] = (p % 32 == j): tensor_scalar(is_equal) with scalar AP =
                # pmod values: pmod[p] = p % 32 -> supply via iota trick:
                # iota_t[:, :1] is 0 for all p. Instead load from host const.
                nc.sync.dma_start(out=pk[:], in_=t_pk[:])
                fold_s = pps.tile([4, 32], f32, space="PSUM", tag="folds")
                fold_q = pps.tile([4, 32], f32, space="PSUM", tag="foldq")
                nc.tensor.matmul(out=fold_s[:], lhsT=acc_s[:], rhs=pk[:],
                                 start=True, stop=True)
                nc.tensor.matmul(out=fold_q[:], lhsT=acc_q[:], rhs=pk[:],
                                 start=True, stop=True)
                ev = pool.tile([4, 64], f32, tag="mfev")
                nc.scalar.activation(out=ev[:, :32], in_=fold_s[:], func=AF.Copy)
                nc.scalar.activation(out=ev[:, 32:], in_=fold_q[:], func=AF.Copy)
                nc.sync.dma_start(out=st_loc[2, :].rearrange("(s k) -> s k", s=4),
                                  in_=ev[:, :32])
                nc.sync.dma_start(out=st_loc[3, :].rearrange("(s k) -> s k", s=4),
                                  in_=ev[:, 32:])
            nc.gpsimd.collective_compute("AllReduce", OP.add, replica_groups=RG,
                                         ins=[st_loc[:]], outs=[st_glob[:]])

        def bn_fold(st_glob, t_bnv, t_w_f, t_w_x, t_fb, count, wpool):
            """returns (wf_scaled, wx_scaled, bias_row) tiles in wpool"""
            p = wpool
            sum_f = p.tile([128, 1], f32, tag="bsf")
            nc.sync.dma_start(out=sum_f[:], in_=st_glob[0, :].rearrange("(p o) -> p o", o=1))
            sq_f = p.tile([128, 1], f32, tag="bqf")
            nc.sync.dma_start(out=sq_f[:], in_=st_glob[1, :].rearrange("(p o) -> p o", o=1))
            sum_x = p.tile([128, 1], f32, tag="bsx")
            nc.sync.dma_start(out=sum_x[:],
                              in_=st_glob[2, :].rearrange("(s k o) -> k s o", s=4, o=1))
            sq_x = p.tile([128, 1], f32, tag="bqx")
            nc.sync.dma_start(out=sq_x[:],
                              in_=st_glob[3, :].rearrange("(s k o) -> k s o", s=4, o=1))
            g_f = p.tile([128, 1], f32, tag="bgf")
            nc.sync.dma_start(out=g_f[:], in_=t_bnv[0, :].rearrange("(p o) -> p o", o=1))
            be_f = p.tile([128, 1], f32, tag="bbf")
            nc.sync.dma_start(out=be_f[:], in_=t_bnv[1, :].rearrange("(p o) -> p o", o=1))
            g_x = p.tile([128, 1], f32, tag="bgx")
            nc.sync.dma_start(out=g_x[:], in_=t_bnv[2, :].rearrange("(p o) -> p o", o=1))
            be_x = p.tile([128, 1], f32, tag="bbx")
            nc.sync.dma_start(out=be_x[:], in_=t_bnv[3, :].rearrange("(p o) -> p o", o=1))

            outs = []
            for (sm, sq, ga, be, t_w, tg) in ((sum_f, sq_f, g_f, be_f, t_w_f, "f"),
                                              (sum_x, sq_x, g_x, be_x, t_w_x, "x")):
                mu = p.tile([128, 1], f32, tag="bmu" + tg)
                nc.vector.tensor_scalar(out=mu[:], in0=sm[:], scalar1=1.0 / count,
                                        scalar2=None, op0=OP.mult)
                var = p.tile([128, 1], f32, tag="bvar" + tg)
                nc.vector.tensor_scalar(out=var[:], in0=sq[:], scalar1=1.0 / count,
                                        scalar2=None, op0=OP.mult)
                mu2 = p.tile([128, 1], f32, tag="bmu2" + tg)
                nc.vector.tensor_tensor(out=mu2[:], in0=mu[:], in1=mu[:], op=OP.mult)
                nc.vector.tensor_tensor(out=var[:], in0=var[:], in1=mu2[:], op=OP.subtract)
                nc.vector.tensor_scalar(out=var[:], in0=var[:], scalar1=EPS,
                                        scalar2=None, op0=OP.add)
                sd = p.tile([128, 1], f32, tag="bsd" + tg)
                nc.scalar.activation(out=sd[:], in_=var[:], func=AF.Sqrt)
                rs = p.tile([128, 1], f32, tag="brs" + tg)
                nc.vector.reciprocal(out=rs[:], in_=sd[:])
                A = p.tile([128, 1], f32, tag="bA" + tg)
                nc.vector.tensor_tensor(out=A[:], in0=rs[:], in1=ga[:], op=OP.mult)
                bc = p.tile([128, 1], f32, tag="bbc" + tg)
                nc.vector.tensor_tensor(out=bc[:], in0=mu[:], in1=A[:], op=OP.mult)
                nc.vector.tensor_tensor(out=bc[:], in0=be[:], in1=bc[:], op=OP.subtract)
                wt = p.tile([128, 128], f32, tag="bwt" + tg)
                nc.sync.dma_start(out=wt[:], in_=t_w[:])
                ws = p.tile([128, 128], f32, tag="bws" + tg)
                nc.scalar.activation(out=ws[:], in_=wt[:], func=AF.Copy, scale=A[:, :1])
                outs.append((wt, ws, bc))
            (wtf, wsf, bcf), (wtx, wsx, bcx) = outs
            with tc.tile_pool(name="bnps", bufs=1, space="PSUM") as bps:
                bp = bps.tile([1, 128], f32, space="PSUM", tag="bp")
                nc.tensor.matmul(out=bp[:], lhsT=bcf[:], rhs=wtf[:], start=True, stop=False)
                nc.tensor.matmul(out=bp[:], lhsT=bcx[:], rhs=wtx[:], start=False, stop=True)
                fb = p.tile([1, 128], f32, tag="bfb")
                nc.sync.dma_start(out=fb[:], in_=t_fb[:])
                brow = p.tile([1, 128], f32, tag="bbrow")
                nc.vector.tensor_tensor(out=brow[:], in0=bp[:], in1=fb[:], op=OP.add)
            return wsf, wsx, brow

        if PHMAX >= 4:
            m_stats(m1r, FPC, st0_loc, st0_glob)
        if PHMAX >= 5:
            wpool0 = tc.alloc_tile_pool(name="w0pool", bufs=1)
            w0fp, w0xp, b0row = bn_fold(st0_glob, t_bn0, t_w0f, t_w0x, t_fb0, COUNT0, wpool0)

        # ================= phase 2: BN0-linear + f_out + T2 =================
        def lin_phase(npart_chunks, t_in_sh, m_r, wfp, wxp, brow, t_out,
                      t2blk_dst, residual, pfx):
            with tc.tile_pool(name=pfx + "sb", bufs=4) as pool, \
                 tc.tile_pool(name=pfx + "ps", bufs=3, space="PSUM") as pps, \
                 tc.tile_pool(name=pfx + "tps", bufs=2, space="PSUM") as tpps:
                for ci in range(npart_chunks):
                    f0 = ci * 128
                    if t2blk_dst is not None:
                        blk = pool.tile([128, 512], bf16, tag="t2blk", name="t2blk")
                    else:
                        blk = None
                    for b in range(B):
                        xt, et = elu(pool, t_in_sh[b, f0:f0 + 128, :], 128, pfx + "e")
                        tp = tpps.tile([128, 128], f32, space="PSUM", tag="tp")
                        nc.tensor.transpose(out=tp[:], in_=et[:], identity=id_t[:])
                        lf = pool.tile([128, 128], f32, tag="lf")
                        nc.scalar.activation(out=lf[:], in_=tp[:], func=AF.Copy)
                        lx = pool.tile([128, 128], f32, tag="lx")
                        nc.sync.dma_start(out=lx[:], in_=m_r[b, :, :, f0:f0 + 128])
                        ps_o = pps.tile([128, 128], f32, space="PSUM", tag="po")
                        nc.tensor.matmul(out=ps_o[:], lhsT=lf[:], rhs=wfp[:],
                                         start=True, stop=False)
                        nc.tensor.matmul(out=ps_o[:], lhsT=lx[:], rhs=wxp[:],
                                         start=False, stop=False)
                        nc.tensor.matmul(out=ps_o[:], lhsT=onesr_t[:], rhs=brow[:],
                                         start=False, stop=True)
                        if residual:
                            ot = pool.tile([128, 128], f32, tag="ot")
                            nc.vector.tensor_tensor(out=ot[:], in0=ps_o[:], in1=xt[:],
                                                    op=OP.add)
                            nc.sync.dma_start(out=t_out[b, f0:f0 + 128, :], in_=ot[:])
                        else:
                            ot = pool.tile([128, 128], f32, tag="ot")
                            nc.scalar.activation(out=ot[:], in_=ps_o[:], func=AF.Copy)
                            nc.sync.dma_start(out=t_out[b, f0:f0 + 128, :], in_=ot[:])
                        if blk is not None:
                            m2 = pool.tile([128, 128], f32, tag="m2")
                            nc.vector.tensor_scalar(out=m2[:], in0=ps_o[:], scalar1=0.0,
                                                    scalar2=None, op0=OP.min)
                            nc.scalar.activation(out=m2[:], in_=m2[:], func=AF.Exp)
                            nc.vector.tensor_scalar(out=m2[:], in0=m2[:], scalar1=1.0,
                                                    scalar2=None, op0=OP.subtract)
                            nc.vector.tensor_tensor(
                                out=blk[:].rearrange("p (s bb k) -> p bb s k", s=4, bb=4)[:, b],
                                in0=ps_o[:].rearrange("p (s k) -> p s k", s=4),
                                in1=m2[:].rearrange("p (s k) -> p s k", s=4),
                                op=OP.max)
                    if blk is not None:
                        nc.sync.dma_start(
                            out=t2blk_dst[f0 * 4:(f0 + 128) * 4, :]
                            .rearrange("(n s) c -> n s c", s=4),
                            in_=blk[:].rearrange("p (s c) -> p s c", s=4))

        if PHMAX >= 5:
            lin_phase(FPC // 128, t_fsh, m1r, w0fp, w0xp, b0row, t_fout, t2sh, False, "l2")
            wpool0.release()

        if PHMAX >= 6:
            nc.gpsimd.collective_compute("AllGather", OP.bypass, replica_groups=RG,
                                         ins=[t2sh[:]], outs=[t2full[:]])

        # ================= phase 3: spbmm2 (DiA) =================
        if PHMAX >= 7:
            seg_phase(NB2, W2, CH2, p3idx, p3rel, p3val, slots2, nsl2,
                      t2full, m2r, NPC, "s3")

        if PHMAX >= 8:
            m_stats(m2r, NPC, st1_loc, st1_glob)
        if PHMAX >= 9:
            wpool1 = tc.alloc_tile_pool(name="w1pool", bufs=1)
            w1fp, w1xp, b1row = bn_fold(st1_glob, t_bn1, t_w1f, t_w1x, t_fb1, COUNT1, wpool1)
            lin_phase(NPC // 128, t_vsh, m2r, w1fp, w1xp, b1row, t_vout, None, True, "l4")
            wpool1.release()
        cpool.release()

    nc.compile()

    in_maps = []
    for c in range(NC):
        m = {
            "vsh": vpad[:, c * NPC:(c + 1) * NPC, :].copy(),
            "fsh": fpad[:, c * FPC:(c + 1) * FPC, :].copy(),
        }
        m.update(consts)
        m.update(meta[c])
        in_maps.append(m)

    trace = bool(int(os.environ.get("KTRACE", "0")))
    res = run_bass_kernel_spmd(nc, in_maps, core_ids=list(range(NC)), trace=trace)
    kernel.last_exec_time_ns = getattr(res, "exec_time_ns", None)

    vout = np.concatenate([res.results[c]["vout"] for c in range(NC)], axis=1)[:, :N]
    fout = np.concatenate([res.results[c]["fout"] for c in range(NC)], axis=1)[:, :F_]
    return vout, fout


# revision 12
# speedup vs baseline: 12.5182x; 3.5975x over previous
import sys

for p in ('/opt/trn_rl_repo', '/root/problem'):
    if p not in sys.path:
        sys.path.insert(0, p)

import numpy as np

# ---- problem constants (hardcoded per contract) ----
B, N, F_, C = 4, 30000, 60000, 128
NNZ = 720000
EPS = 1e-5
NC = 8
NPC = 3840                  # nodes per core (8*3840 = 30720 >= 30000), /128
FPC = 7552                  # faces per core (8*7552 = 60416 >= 60000), /128
NR1 = FPC * 4               # dest rows per core, phase 1
NR2 = NPC * 4               # dest rows per core, phase 3
NB1 = NR1 // 128            # 236
NB2 = NR2 // 128            # 120
T1_ROWS = NC * NPC * 4      # 122880
T2_ROWS = NC * FPC * 4      # 241664
W1 = 4
W2 = 8
WIN = 32768
CH1 = 16                    # slots per gather chunk (2048 entries), phase 1
CH2 = 16                    # slots per gather chunk (2048 entries), phase 3


def _schedule(dst_core, dst_rel, src_row, vals, nblocks, nwin):
    blk = dst_rel // 128
    rel = (dst_rel % 128).astype(np.float32)
    win = src_row // WIN
    idx16 = (src_row - win * WIN).astype(np.int16)

    counts = np.zeros((NC, nblocks, nwin), dtype=np.int64)
    np.add.at(counts, (dst_core, blk, win), 1)
    need = (counts.max(axis=0) + 127) // 128
    need = np.maximum(need, 1)
    slots = [[w for w in range(nwin) for _ in range(int(need[b, w]))]
             for b in range(nblocks)]
    n_slots_w = [int(need[:, w].sum()) for w in range(nwin)]
    base_w = []
    for w in range(nwin):
        base = np.zeros(nblocks + 1, dtype=np.int64)
        pos = 0
        for b in range(nblocks):
            base[b] = pos
            pos += int(need[b, w]) * 128
        base[nblocks] = pos
        base_w.append(base)

    per_core = []
    for c in range(NC):
        m = dst_core == c
        cb, cw = blk[m], win[m]
        order = np.lexsort((cw, cb))
        cb, cw = cb[order], cw[order]
        cr, cv, ci = rel[m][order], vals[m][order], idx16[m][order]
        streams = []
        for w in range(nwin):
            ns = n_slots_w[w]
            si = np.zeros(ns * 128, dtype=np.int16)
            sr = np.zeros(ns * 128, dtype=np.float32)
            sv = np.zeros(ns * 128, dtype=np.float32)
            sel = cw == w
            eb = cb[sel]
            off = np.zeros(len(eb), dtype=np.int64)
            if len(eb):
                change = np.flatnonzero(np.diff(eb) != 0) + 1
                starts = np.concatenate(([0], change))
                lens = np.diff(np.concatenate((starts, [len(eb)])))
                off = np.arange(len(eb)) - np.repeat(starts, lens)
            dst = base_w[w][eb] + off
            si[dst] = ci[sel]
            sr[dst] = cr[sel]
            sv[dst] = cv[sel]
            streams.append((si, sr, sv))
        per_core.append(streams)
    return slots, n_slots_w, per_core


def _wrap16(si, n_slots, chunk_slots):
    ns_pad = max(((n_slots + chunk_slots - 1) // chunk_slots) * chunk_slots, chunk_slots)
    full = np.zeros(ns_pad * 128, dtype=np.int16)
    full[:len(si)] = si
    cols = ns_pad * 8
    w = np.zeros((128, cols), dtype=np.int16)
    idx = np.arange(ns_pad * 128)
    w[idx % 16, idx // 16] = full
    for g in range(1, 8):
        w[g * 16:(g + 1) * 16] = w[:16]
    return w, ns_pad


def _slotpack(arr, n_slots):
    out = np.zeros((128, max(n_slots, 1)), dtype=np.float32)
    if n_slots:
        out[:, :n_slots] = arr.reshape(n_slots, 128).T
    return out


def kernel(Di_rows, Di_cols, Di_vals, DiA_rows, DiA_cols, DiA_vals, v, f,
           bn0_gamma, bn0_beta, fc0_w, fc0_b, bn1_gamma, bn1_beta, fc1_w, fc1_b):
    import concourse.bass as bass
    import concourse.bacc as bacc
    import concourse.tile as tile
    from concourse import mybir
    from concourse.bass_utils import run_bass_kernel_spmd

    f32 = mybir.dt.float32
    bf16 = mybir.dt.bfloat16
    AF = mybir.ActivationFunctionType
    OP = mybir.AluOpType

    Di_rows = np.asarray(Di_rows); Di_cols = np.asarray(Di_cols)
    Di_vals = np.asarray(Di_vals, dtype=np.float32)
    DiA_rows = np.asarray(DiA_rows); DiA_cols = np.asarray(DiA_cols)
    DiA_vals = np.asarray(DiA_vals, dtype=np.float32)
    v = np.asarray(v, dtype=np.float32); f = np.asarray(f, dtype=np.float32)
    bn0_gamma = np.asarray(bn0_gamma, dtype=np.float32)
    bn0_beta = np.asarray(bn0_beta, dtype=np.float32)
    fc0_w = np.asarray(fc0_w, dtype=np.float32)
    fc0_b = np.asarray(fc0_b, dtype=np.float32)
    bn1_gamma = np.asarray(bn1_gamma, dtype=np.float32)
    bn1_beta = np.asarray(bn1_beta, dtype=np.float32)
    fc1_w = np.asarray(fc1_w, dtype=np.float32)
    fc1_b = np.asarray(fc1_b, dtype=np.float32)

    # ---------------- host-side index preprocessing ----------------
    r = Di_rows.astype(np.int64)
    fc = r // 4
    d1_core = fc // FPC
    d1_rel = 4 * (fc - d1_core * FPC) + (r % 4)
    c4 = Di_cols.astype(np.int64)
    nd = c4 // 4
    scr = nd // NPC
    t1row = scr * (NPC * 4) + 4 * (nd - scr * NPC) + (c4 % 4)
    slots1, nsl1, pc1 = _schedule(d1_core, d1_rel, t1row, Di_vals, NB1, W1)

    r2 = DiA_rows.astype(np.int64)
    nd2 = r2 // 4
    d2_core = nd2 // NPC
    d2_rel = 4 * (nd2 - d2_core * NPC) + (r2 % 4)
    c42 = DiA_cols.astype(np.int64)
    fc2 = c42 // 4
    sc2 = fc2 // FPC
    t2row = sc2 * (FPC * 4) + 4 * (fc2 - sc2 * FPC) + (c42 % 4)
    slots2, nsl2, pc2 = _schedule(d2_core, d2_rel, t2row, DiA_vals, NB2, W2)

    meta = []
    ncols1 = [None] * W1
    ncols2 = [None] * W2
    for c in range(NC):
        m = {}
        for w in range(W1):
            si, sr, sv = pc1[c][w]
            wi, ns_pad = _wrap16(si, nsl1[w], CH1)
            ncols1[w] = ns_pad
            m[f"p1idx{w}"] = wi
            m[f"p1rel{w}"] = _slotpack(sr, nsl1[w])
            m[f"p1val{w}"] = _slotpack(sv, nsl1[w])
        for w in range(W2):
            si, sr, sv = pc2[c][w]
            wi, ns_pad = _wrap16(si, nsl2[w], CH2)
            ncols2[w] = ns_pad
            m[f"p3idx{w}"] = wi
            m[f"p3rel{w}"] = _slotpack(sr, nsl2[w])
            m[f"p3val{w}"] = _slotpack(sv, nsl2[w])
        meta.append(m)

    vpad = np.zeros((B, NC * NPC, C), dtype=np.float32)
    vpad[:, :N] = v
    fpad = np.zeros((B, NC * FPC, C), dtype=np.float32)
    fpad[:, :F_] = f

    kk = np.arange(32); ss = np.arange(4)
    perm_ks = (128 + 32 * ss[None, :] + kk[:, None]).reshape(-1)
    W0T = fc0_w.T
    W1T = fc1_w.T
    consts = {
        "w0f": W0T[:128].copy(), "w0x": W0T[perm_ks].copy(),
        "w1f": W1T[:128].copy(), "w1x": W1T[perm_ks].copy(),
        "bn0v": np.stack([bn0_gamma[:128], bn0_beta[:128],
                          bn0_gamma[perm_ks], bn0_beta[perm_ks]]),
        "bn1v": np.stack([bn1_gamma[:128], bn1_beta[:128],
                          bn1_gamma[perm_ks], bn1_beta[perm_ks]]),
        "fb0": fc0_b.reshape(1, 128).copy(), "fb1": fc1_b.reshape(1, 128).copy(),
        "iota": np.tile(np.arange(128, dtype=np.float32), (128, 1)),
        "ident": np.eye(128, dtype=np.float32),
        "onesc": np.ones((128, 1), dtype=np.float32),
        "onesr": np.ones((1, 128), dtype=np.float32),
        "pk32": (np.arange(128)[:, None] % 32 == np.arange(32)[None, :]).astype(np.float32),
    }

    # ---------------- build the SPMD program ----------------
    nc = bacc.Bacc("TRN2", target_bir_lowering=False, debug=False, num_devices=NC)

    def din(name, shape, dtype=f32):
        return nc.declare_dram_parameter(name, list(shape), dtype, isOutput=False)

    t_vsh = din("vsh", [B, NPC, C])
    t_fsh = din("fsh", [B, FPC, C])
    t_w0f = din("w0f", [128, 128]); t_w0x = din("w0x", [128, 128])
    t_w1f = din("w1f", [128, 128]); t_w1x = din("w1x", [128, 128])
    t_bn0 = din("bn0v", [4, 128]); t_bn1 = din("bn1v", [4, 128])
    t_fb0 = din("fb0", [1, 128]); t_fb1 = din("fb1", [1, 128])
    t_iota = din("iota", [128, 128]); t_id = din("ident", [128, 128])
    t_ones = din("onesc", [128, 1]); t_onesr = din("onesr", [1, 128])
    t_pk = din("pk32", [128, 32])
    p1idx = [din(f"p1idx{w}", [128, ncols1[w] * 8], mybir.dt.int16) for w in range(W1)]
    p1rel = [din(f"p1rel{w}", [128, max(nsl1[w], 1)]) for w in range(W1)]
    p1val = [din(f"p1val{w}", [128, max(nsl1[w], 1)]) for w in range(W1)]
    p3idx = [din(f"p3idx{w}", [128, ncols2[w] * 8], mybir.dt.int16) for w in range(W2)]
    p3rel = [din(f"p3rel{w}", [128, max(nsl2[w], 1)]) for w in range(W2)]
    p3val = [din(f"p3val{w}", [128, max(nsl2[w], 1)]) for w in range(W2)]

    t_vout = nc.declare_dram_parameter("vout", [B, NPC, C], f32, isOutput=True)
    t_fout = nc.declare_dram_parameter("fout", [B, FPC, C], f32, isOutput=True)

    t1sh = nc.dram_tensor("t1sh", [NPC * 4, 128], bf16)
    t1full = nc.dram_tensor("t1full", [T1_ROWS, 128], bf16, addr_space="Shared")
    t2sh = nc.dram_tensor("t2sh", [FPC * 4, 128], bf16)
    t2full = nc.dram_tensor("t2full", [T2_ROWS, 128], bf16, addr_space="Shared")
    m1r = nc.dram_tensor("m1r", [4, 32, 4, FPC], f32)
    m2r = nc.dram_tensor("m2r", [4, 32, 4, NPC], f32)
    st0_loc = nc.dram_tensor("st0_loc", [4, 128], f32)
    st0_glob = nc.dram_tensor("st0_glob", [4, 128], f32)
    st1_loc = nc.dram_tensor("st1_loc", [4, 128], f32)
    st1_glob = nc.dram_tensor("st1_glob", [4, 128], f32)

    import os
    PHMAX = int(os.environ.get("KPHASES", "9"))
    KSEG = int(os.environ.get("KSEG", "3"))
    COUNT0 = float(B * F_)
    COUNT1 = float(B * N)
    RG = [list(range(NC))]

    with tile.TileContext(nc) as tc:
        cpool = tc.alloc_tile_pool(name="const", bufs=1)
        iota_t = cpool.tile([128, 128], f32)
        nc.sync.dma_start(out=iota_t[:], in_=t_iota[:])
        id_t = cpool.tile([128, 128], f32)
        nc.sync.dma_start(out=id_t[:], in_=t_id[:])
        ones_t = cpool.tile([128, 1], f32)
        nc.sync.dma_start(out=ones_t[:], in_=t_ones[:])
        onesr_t = cpool.tile([1, 128], f32)
        nc.sync.dma_start(out=onesr_t[:], in_=t_onesr[:])

        def elu(pool, src_ap, Fdim, tag):
            """returns (x_tile, elu_tile) both [128, Fdim]"""
            xt = pool.tile([128, Fdim], f32, tag=tag + "x")
            nc.sync.dma_start(out=xt[:], in_=src_ap)
            mt = pool.tile([128, Fdim], f32, tag=tag + "m")
            nc.vector.tensor_scalar(out=mt[:], in0=xt[:], scalar1=0.0,
                                    scalar2=None, op0=OP.min)
            nc.scalar.activation(out=mt[:], in_=mt[:], func=AF.Exp)
            nc.vector.tensor_scalar(out=mt[:], in0=mt[:], scalar1=1.0,
                                    scalar2=None, op0=OP.subtract)
            ot = pool.tile([128, Fdim], f32, tag=tag + "o")
            nc.vector.tensor_tensor(out=ot[:], in0=xt[:], in1=mt[:], op=OP.max)
            return xt, ot

        # ================= phase 0: T1 build + input stats =================
        with tc.tile_pool(name="p0", bufs=3) as pool, \
             tc.tile_pool(name="p0ps", bufs=1, space="PSUM") as pps:
            xs_ps = pps.tile([128, 1], f32, space="PSUM", tag="xs")
            xq_ps = pps.tile([128, 1], f32, space="PSUM", tag="xq")
            fs_ps = pps.tile([128, 1], f32, space="PSUM", tag="fs")
            fq_ps = pps.tile([128, 1], f32, space="PSUM", tag="fq")
            n_nch = NPC // 128
            step = 0
            last_step = n_nch * B - 1
            for ci in range(n_nch):
                n0 = ci * 128
                blk = pool.tile([128, 512], bf16, tag="t1blk")
                for b in range(B):
                    _, et = elu(pool, t_vsh[b, n0:n0 + 128, :], 128, "v")
                    # scatter into (s, b, k) positions of blk
                    nc.scalar.activation(
                        out=blk[:].rearrange("p (s bb k) -> p bb s k", s=4, bb=4)[:, b],
                        in_=et[:].rearrange("p (s k) -> p s k", s=4),
                        func=AF.Copy)
                    sq = pool.tile([128, 128], f32, tag="sqx")
                    nc.scalar.activation(out=sq[:], in_=et[:], func=AF.Square)
                    nc.tensor.matmul(out=xs_ps[:], lhsT=et[:], rhs=ones_t[:],
                                     start=(step == 0), stop=(step == last_step))
                    nc.tensor.matmul(out=xq_ps[:], lhsT=sq[:], rhs=ones_t[:],
                                     start=(step == 0), stop=(step == last_step))
                    step += 1
                nc.sync.dma_start(
                    out=t1sh[n0 * 4:(n0 + 128) * 4, :].rearrange("(n s) c -> n s c", s=4),
                    in_=blk[:].rearrange("p (s c) -> p s c", s=4))
            # f stats
            n_fch = FPC // 128
            step = 0
            last_step = n_fch * B - 1
            for ci in range(n_fch):
                f0 = ci * 128
                for b in range(B):
                    _, et = elu(pool, t_fsh[b, f0:f0 + 128, :], 128, "f")
                    sq = pool.tile([128, 128], f32, tag="sqf")
                    nc.scalar.activation(out=sq[:], in_=et[:], func=AF.Square)
                    nc.tensor.matmul(out=fs_ps[:], lhsT=et[:], rhs=ones_t[:],
                                     start=(step == 0), stop=(step == last_step))
                    nc.tensor.matmul(out=fq_ps[:], lhsT=sq[:], rhs=ones_t[:],
                                     start=(step == 0), stop=(step == last_step))
                    step += 1
            st = pool.tile([128, 4], f32, tag="stev")
            nc.scalar.activation(out=st[:, 0:1], in_=fs_ps[:], func=AF.Copy)
            nc.scalar.activation(out=st[:, 1:2], in_=fq_ps[:], func=AF.Copy)
            nc.scalar.activation(out=st[:, 2:3], in_=xs_ps[:], func=AF.Copy)
            nc.scalar.activation(out=st[:, 3:4], in_=xq_ps[:], func=AF.Copy)
            nc.sync.dma_start(out=st0_loc[0, :].rearrange("(p o) -> p o", o=1), in_=st[:, 0:1])
            nc.sync.dma_start(out=st0_loc[1, :].rearrange("(p o) -> p o", o=1), in_=st[:, 1:2])
            nc.sync.dma_start(out=st1_loc[0, :].rearrange("(p o) -> p o", o=1), in_=st[:, 2:3])
            nc.sync.dma_start(out=st1_loc[1, :].rearrange("(p o) -> p o", o=1), in_=st[:, 3:4])

        if PHMAX >= 2:
            nc.gpsimd.collective_compute("AllGather", OP.bypass, replica_groups=RG,
                                         ins=[t1sh[:]], outs=[t1full[:]])

        # ================= segment-sum phases =================
        def seg_phase(nblocks, nwin, chs, idxs_d, rel_d, val_d, slots, nsl,
                      table, out_r, out_fdim, pfx):
            with tc.tile_pool(name=pfx + "meta", bufs=1) as mpool, \
                 tc.tile_pool(name=pfx + "sb", bufs=4) as pool, \
                 tc.tile_pool(name=pfx + "g", bufs=3) as gpool, \
                 tc.tile_pool(name=pfx + "ps", bufs=4, space="PSUM") as pps:
                idx_ts, rel_ts, val_ts = [], [], []
                for w in range(nwin):
                    it = mpool.tile([128, idxs_d[w].shape[1]], mybir.dt.int16,
                                    tag=f"mi{w}")
                    nc.sync.dma_start(out=it[:], in_=idxs_d[w][:])
                    idx_ts.append(it)
                    rt = mpool.tile([128, rel_d[w].shape[1]], f32, tag=f"mr{w}")
                    nc.sync.dma_start(out=rt[:], in_=rel_d[w][:])
                    rel_ts.append(rt)
                    vt = mpool.tile([128, val_d[w].shape[1]], f32, tag=f"mv{w}")
                    nc.sync.dma_start(out=vt[:], in_=val_d[w][:])
                    val_ts.append(vt)
                cur_chunk = [None] * nwin
                cur_ci = [-1] * nwin
                gslot = [0] * nwin

                def ensure_chunk(w, ci):
                    if cur_ci[w] == ci:
                        return
                    g = gpool.tile([128, chs, 128], bf16, tag=f"g{w}")
                    if KSEG != 2:
                        nc.gpsimd.dma_gather(
                            out_ap=g[:], in_ap=table[w * WIN:, :],
                            idxs_ap=idx_ts[w][:, ci * chs * 8:(ci + 1) * chs * 8],
                            num_idxs=chs * 128, num_idxs_reg=chs * 128, elem_size=128,
                            single_packet=False)
                    else:
                        nc.vector.memset(g[:, 0, :], 0.0)
                    cur_chunk[w] = g
                    cur_ci[w] = ci

                for b in range(nblocks):
                    if KSEG == 1:
                        for w in set(slots[b]):
                            gs = gslot[w]
                            ensure_chunk(w, gs // chs)
                        for w in slots[b]:
                            gslot[w] += 1
                        continue
                    ps = pps.tile([128, 128], f32, space="PSUM", tag="seg")
                    sl = slots[b]
                    for i, w in enumerate(sl):
                        gs = gslot[w]
                        ensure_chunk(w, gs // chs)
                        j = gs % chs
                        sel = pool.tile([128, 128], bf16, tag="sel")
                        nc.vector.tensor_scalar(
                            out=sel[:], in0=iota_t[:],
                            scalar1=rel_ts[w][:, gs:gs + 1],
                            scalar2=val_ts[w][:, gs:gs + 1],
                            op0=OP.is_equal, op1=OP.mult)
                        nc.tensor.matmul(out=ps[:], lhsT=cur_chunk[w][:, j, :],
                                         rhs=sel[:], start=(i == 0),
                                         stop=(i == len(sl) - 1))
                        gslot[w] += 1
                    mt = pool.tile([128, 128], f32, tag="mev")
                    nc.scalar.activation(
                        out=mt[:].rearrange("p (s fl) -> p s fl", s=4),
                        in_=ps[:].rearrange("p (fl s) -> p s fl", s=4),
                        func=AF.Copy)
                    f0 = b * 32
                    nc.sync.dma_start(
                        out=out_r[:, :, :, f0:f0 + 32],
                        in_=mt[:].rearrange("p (s fl) -> p s fl", s=4))

        if PHMAX >= 3:
            seg_phase(NB1, W1, CH1, p1idx, p1rel, p1val, slots1, nsl1,
                      t1full, m1r, FPC, "s1")

        # ---- stats over m1r + allreduce + W0' build ----
        def m_stats(src, fdim, st_loc, st_glob):
            with tc.tile_pool(name="mst", bufs=3) as pool, \
                 tc.tile_pool(name="mstps", bufs=1, space="PSUM") as pps:
                acc_s = pool.tile([128, 4], f32, tag="accs")
                acc_q = pool.tile([128, 4], f32, tag="accq")
                nc.vector.memset(acc_s[:], 0.0)
                nc.vector.memset(acc_q[:], 0.0)
                nch = fdim // 512
                rem = fdim - nch * 512
                spans = [(i * 512, 512) for i in range(nch)]
                if rem:
                    spans.append((nch * 512, rem))
                for (f0, ln) in spans:
                    xt = pool.tile([128, 4, 512], f32, tag="mstx")
                    nc.sync.dma_start(out=xt[:, :, :ln], in_=src[:, :, :, f0:f0 + ln])
                    sq = pool.tile([128, 4, 512], f32, tag="mstq")
                    nc.scalar.activation(out=sq[:, :, :ln], in_=xt[:, :, :ln],
                                         func=AF.Square)
                    for s in range(4):
                        t1 = pool.tile([128, 1], f32, tag="mr1")
                        nc.vector.reduce_sum(out=t1[:], in_=xt[:, s, :ln], axis=mybir.AxisListType.X)
                        nc.vector.tensor_tensor(out=acc_s[:, s:s + 1],
                                                in0=acc_s[:, s:s + 1], in1=t1[:],
                                                op=OP.add)
                        t2 = pool.tile([128, 1], f32, tag="mr2")
                        nc.vector.reduce_sum(out=t2[:], in_=sq[:, s, :ln], axis=mybir.AxisListType.X)
                        nc.vector.tensor_tensor(out=acc_q[:, s:s + 1],
                                                in0=acc_q[:, s:s + 1], in1=t2[:],
                                                op=OP.add)
                # fold b: out[s, k] = sum_b acc[(b,k), s] via matmul with Pk
                pk = pool.tile([128, 32], f32, tag="pk")
                # build Pk = (iota32 == k_index): k index per partition = p % 32
                # use iota columns 0..31 compared to (p%32): precompute on host? use
                # iota_t[:, :32] == pmod tile: simplest: DMA from host const? reuse:
                # Pk[p, j] = (p % 32 == j): tensor_scalar(is_equal) with scalar AP =
                # pmod values: pmod[p] = p % 32 -> supply via iota trick:
                # iota_t[:, :1] is 0 for all p. Instead load from host const.
                nc.sync.dma_start(out=pk[:], in_=t_pk[:])
                fold_s = pps.tile([4, 32], f32, space="PSUM", tag="folds")
                fold_q = pps.tile([4, 32], f32, space="PSUM", tag="foldq")
                nc.tensor.matmul(out=fold_s[:], lhsT=acc_s[:], rhs=pk[:],
                                 start=True, stop=True)
                nc.tensor.matmul(out=fold_q[:], lhsT=acc_q[:], rhs=pk[:],
                                 start=True, stop=True)
                ev = pool.tile([4, 64], f32, tag="mfev")
                nc.scalar.activation(out=ev[:, :32], in_=fold_s[:], func=AF.Copy)
                nc.scalar.activation(out=ev[:, 32:], in_=fold_q[:], func=AF.Copy)
                nc.sync.dma_start(out=st_loc[2, :].rearrange("(s k) -> s k", s=4),
                                  in_=ev[:, :32])
                nc.sync.dma_start(out=st_loc[3, :].rearrange("(s k) -> s k", s=4),
                                  in_=ev[:, 32:])
            nc.gpsimd.collective_compute("AllReduce", OP.add, replica_groups=RG,
                                         ins=[st_loc[:]], outs=[st_glob[:]])

        def bn_fold(st_glob, t_bnv, t_w_f, t_w_x, t_fb, count, wpool):
            """returns (wf_scaled, wx_scaled, bias_row) tiles in wpool"""
            p = wpool
            sum_f = p.tile([128, 1], f32, tag="bsf")
            nc.sync.dma_start(out=sum_f[:], in_=st_glob[0, :].rearrange("(p o) -> p o", o=1))
            sq_f = p.tile([128, 1], f32, tag="bqf")
            nc.sync.dma_start(out=sq_f[:], in_=st_glob[1, :].rearrange("(p o) -> p o", o=1))
            sum_x = p.tile([128, 1], f32, tag="bsx")
            nc.sync.dma_start(out=sum_x[:],
                              in_=st_glob[2, :].rearrange("(s k o) -> k s o", s=4, o=1))
            sq_x = p.tile([128, 1], f32, tag="bqx")
            nc.sync.dma_start(out=sq_x[:],
                              in_=st_glob[3, :].rearrange("(s k o) -> k s o", s=4, o=1))
            g_f = p.tile([128, 1], f32, tag="bgf")
            nc.sync.dma_start(out=g_f[:], in_=t_bnv[0, :].rearrange("(p o) -> p o", o=1))
            be_f = p.tile([128, 1], f32, tag="bbf")
            nc.sync.dma_start(out=be_f[:], in_=t_bnv[1, :].rearrange("(p o) -> p o", o=1))
            g_x = p.tile([128, 1], f32, tag="bgx")
            nc.sync.dma_start(out=g_x[:], in_=t_bnv[2, :].rearrange("(p o) -> p o", o=1))
            be_x = p.tile([128, 1], f32, tag="bbx")
            nc.sync.dma_start(out=be_x[:], in_=t_bnv[3, :].rearrange("(p o) -> p o", o=1))

            outs = []
            for (sm, sq, ga, be, t_w, tg) in ((sum_f, sq_f, g_f, be_f, t_w_f, "f"),
                                              (sum_x, sq_x, g_x, be_x, t_w_x, "x")):
                mu = p.tile([128, 1], f32, tag="bmu" + tg)
                nc.vector.tensor_scalar(out=mu[:], in0=sm[:], scalar1=1.0 / count,
                                        scalar2=None, op0=OP.mult)
                var = p.tile([128, 1], f32, tag="bvar" + tg)
                nc.vector.tensor_scalar(out=var[:], in0=sq[:], scalar1=1.0 / count,
                                        scalar2=None, op0=OP.mult)
                mu2 = p.tile([128, 1], f32, tag="bmu2" + tg)
                nc.vector.tensor_tensor(out=mu2[:], in0=mu[:], in1=mu[:], op=OP.mult)
                nc.vector.tensor_tensor(out=var[:], in0=var[:], in1=mu2[:], op=OP.subtract)
                nc.vector.tensor_scalar(out=var[:], in0=var[:], scalar1=EPS,
                                        scalar2=None, op0=OP.add)
                sd = p.tile([128, 1], f32, tag="bsd" + tg)
                nc.scalar.activation(out=sd[:], in_=var[:], func=AF.Sqrt)
                rs = p.tile([128, 1], f32, tag="brs" + tg)
                nc.vector.reciprocal(out=rs[:], in_=sd[:])
                A = p.tile([128, 1], f32, tag="bA" + tg)
                nc.vector.tensor_tensor(out=A[:], in0=rs[:], in1=ga[:], op=OP.mult)
                bc = p.tile([128, 1], f32, tag="bbc" + tg)
                nc.vector.tensor_tensor(out=bc[:], in0=mu[:], in1=A[:], op=OP.mult)
                nc.vector.tensor_tensor(out=bc[:], in0=be[:], in1=bc[:], op=OP.subtract)
                wt = p.tile([128, 128], f32, tag="bwt" + tg)
                nc.sync.dma_start(out=wt[:], in_=t_w[:])
                ws = p.tile([128, 128], f32, tag="bws" + tg)
                nc.scalar.activation(out=ws[:], in_=wt[:], func=AF.Copy, scale=A[:, :1])
                outs.append((wt, ws, bc))
            (wtf, wsf, bcf), (wtx, wsx, bcx) = outs
            with tc.tile_pool(name="bnps", bufs=1, space="PSUM") as bps:
                bp = bps.tile([1, 128], f32, space="PSUM", tag="bp")
                nc.tensor.matmul(out=bp[:], lhsT=bcf[:], rhs=wtf[:], start=True, stop=False)
                nc.tensor.matmul(out=bp[:], lhsT=bcx[:], rhs=wtx[:], start=False, stop=True)
                fb = p.tile([1, 128], f32, tag="bfb")
                nc.sync.dma_start(out=fb[:], in_=t_fb[:])
                brow = p.tile([1, 128], f32, tag="bbrow")
                nc.vector.tensor_tensor(out=brow[:], in0=bp[:], in1=fb[:], op=OP.add)
            return wsf, wsx, brow

        if PHMAX >= 4:
            m_stats(m1r, FPC, st0_loc, st0_glob)
        if PHMAX >= 5:
            wpool0 = tc.alloc_tile_pool(name="w0pool", bufs=1)
            w0fp, w0xp, b0row = bn_fold(st0_glob, t_bn0, t_w0f, t_w0x, t_fb0, COUNT0, wpool0)

        # ================= phase 2: BN0-linear + f_out + T2 =================
        def lin_phase(npart_chunks, t_in_sh, m_r, wfp, wxp, brow, t_out,
                      t2blk_dst, residual, pfx):
            with tc.tile_pool(name=pfx + "sb", bufs=4) as pool, \
                 tc.tile_pool(name=pfx + "ps", bufs=3, space="PSUM") as pps, \
                 tc.tile_pool(name=pfx + "tps", bufs=2, space="PSUM") as tpps:
                for ci in range(npart_chunks):
                    f0 = ci * 128
                    if t2blk_dst is not None:
                        blk = pool.tile([128, 512], bf16, tag="t2blk", name="t2blk")
                    else:
                        blk = None
                    for b in range(B):
                        xt, et = elu(pool, t_in_sh[b, f0:f0 + 128, :], 128, pfx + "e")
                        tp = tpps.tile([128, 128], f32, space="PSUM", tag="tp")
                        nc.tensor.transpose(out=tp[:], in_=et[:], identity=id_t[:])
                        lf = pool.tile([128, 128], f32, tag="lf")
                        nc.scalar.activation(out=lf[:], in_=tp[:], func=AF.Copy)
                        lx = pool.tile([128, 128], f32, tag="lx")
                        nc.sync.dma_start(out=lx[:], in_=m_r[b, :, :, f0:f0 + 128])
                        ps_o = pps.tile([128, 128], f32, space="PSUM", tag="po")
                        nc.tensor.matmul(out=ps_o[:], lhsT=lf[:], rhs=wfp[:],
                                         start=True, stop=False)
                        nc.tensor.matmul(out=ps_o[:], lhsT=lx[:], rhs=wxp[:],
                                         start=False, stop=False)
                        nc.tensor.matmul(out=ps_o[:], lhsT=onesr_t[:], rhs=brow[:],
                                         start=False, stop=True)
                        if residual:
                            ot = pool.tile([128, 128], f32, tag="ot")
                            nc.vector.tensor_tensor(out=ot[:], in0=ps_o[:], in1=xt[:],
                                                    op=OP.add)
                            nc.sync.dma_start(out=t_out[b, f0:f0 + 128, :], in_=ot[:])
                        else:
                            ot = pool.tile([128, 128], f32, tag="ot")
                            nc.scalar.activation(out=ot[:], in_=ps_o[:], func=AF.Copy)
                            nc.sync.dma_start(out=t_out[b, f0:f0 + 128, :], in_=ot[:])
                        if blk is not None:
                            m2 = pool.tile([128, 128], f32, tag="m2")
                            nc.vector.tensor_scalar(out=m2[:], in0=ps_o[:], scalar1=0.0,
                                                    scalar2=None, op0=OP.min)
                            nc.scalar.activation(out=m2[:], in_=m2[:], func=AF.Exp)
                            nc.vector.tensor_scalar(out=m2[:], in0=m2[:], scalar1=1.0,
                                                    scalar2=None, op0=OP.subtract)
                            nc.vector.tensor_tensor(
                                out=blk[:].rearrange("p (s bb k) -> p bb s k", s=4, bb=4)[:, b],
                                in0=ps_o[:].rearrange("p (s k) -> p s k", s=4),
                                in1=m2[:].rearrange("p (s k) -> p s k", s=4),
                                op=OP.max)
                    if blk is not None:
                        nc.sync.dma_start(
                            out=t2blk_dst[f0 * 4:(f0 + 128) * 4, :]
                            .rearrange("(n s) c -> n s c", s=4),
                            in_=blk[:].rearrange("p (s c) -> p s c", s=4))

        if PHMAX >= 5:
            lin_phase(FPC // 128, t_fsh, m1r, w0fp, w0xp, b0row, t_fout, t2sh, False, "l2")
            wpool0.release()

        if PHMAX >= 6:
            nc.gpsimd.collective_compute("AllGather", OP.bypass, replica_groups=RG,
                                         ins=[t2sh[:]], outs=[t2full[:]])

        # ================= phase 3: spbmm2 (DiA) =================
        if PHMAX >= 7:
            seg_phase(NB2, W2, CH2, p3idx, p3rel, p3val, slots2, nsl2,
                      t2full, m2r, NPC, "s3")

        if PHMAX >= 8:
            m_stats(m2r, NPC, st1_loc, st1_glob)
        if PHMAX >= 9:
            wpool1 = tc.alloc_tile_pool(name="w1pool", bufs=1)
            w1fp, w1xp, b1row = bn_fold(st1_glob, t_bn1, t_w1f, t_w1x, t_fb1, COUNT1, wpool1)
            lin_phase(NPC // 128, t_vsh, m2r, w1fp, w1xp, b1row, t_vout, None, True, "l4")
            wpool1.release()
        cpool.release()

    nc.compile()

    in_maps = []
    for c in range(NC):
        m = {
            "vsh": vpad[:, c * NPC:(c + 1) * NPC, :].copy(),
            "fsh": fpad[:, c * FPC:(c + 1) * FPC, :].copy(),
        }
        m.update(consts)
        m.update(meta[c])
        in_maps.append(m)

    trace = bool(int(os.environ.get("KTRACE", "0")))
    res = run_bass_kernel_spmd(nc, in_maps, core_ids=list(range(NC)), trace=trace)
    kernel.last_exec_time_ns = getattr(res, "exec_time_ns", None)

    vout = np.concatenate([res.results[c]["vout"] for c in range(NC)], axis=1)[:, :N]
    fout = np.concatenate([res.results[c]["fout"] for c in range(NC)], axis=1)[:, :F_]
    return vout, fout
